# revision 18
# baseline (speedup 1.0000x reference)
"""Fused multi-head attention (qkv + RoPE + softmax + proj) for TRN2, 8 cores.

Sharding: core c -> batch b=c//2, head group hg=c%2 (8 of 16 heads).
Data-parallel over B (4), 2-way tensor-parallel over heads.
Host unshard: out[b] = partial[2b] + partial[2b+1] + b_proj.

v4: all matmul operands bf16 (1 cyc/col @2.4GHz).  The attention loop is
gated by the ScalarE softmax exp at ~(1024+340)/1.2GHz ~= 1.12us per key
tile; everything else is paced to hide under it:
  head:  k^T (+rope; the rot matmul is delayed one tile so the PE never
         waits on the psum->sbuf copy), v, q^T chunk 0.
  loop:  scores pair (tile_position quadrants) -> exp -> attn@v pair, plus
         a filler queue popped once per ki at 1-matmul granularity:
         remaining q^T tiles, proj tiles, and the block-boundary softmax
         normalization (reciprocal on DVE, partition-broadcast via a
         [1,64] ones matmul, psum x psum multiply into aoT on DVE).
PSUM budget: scores 2x2 banks, attn@v accumulators 2 (single-buffered),
filler accum 1, rot/broadcast 1.
"""

import sys

if "/opt/trn_rl_repo" not in sys.path:
    sys.path.insert(0, "/opt/trn_rl_repo")

import numpy as np
from contextlib import ExitStack

B, N, C, H, D = 4, 2048, 1024, 16, 64
NCORES = 8
P = 128
DH = 512          # per-core head channels (8 heads x 64)
CT = C // P       # 8 contraction tiles for qkv
DHT = DH // P     # 4 partition tiles of qT/kT/aoT
NT = N // P       # 16 n tiles
NCH = N // 512    # 4 n chunks of 512
KT = N // P       # 16 key tiles

_CACHE = {}


def _emit(nc, tc, mybir, bass, tile):
    F32 = mybir.dt.float32
    BF16 = mybir.dt.bfloat16
    FP16 = mybir.dt.float16
    Exp = mybir.ActivationFunctionType.Exp

    xT = nc.dram_tensor("xT", [C, N], BF16, kind="ExternalInput").ap()
    wq = nc.dram_tensor("wq", [C, DH], BF16, kind="ExternalInput").ap()
    wk = nc.dram_tensor("wk", [C, DH], BF16, kind="ExternalInput").ap()
    wv = nc.dram_tensor("wv", [C, DH], BF16, kind="ExternalInput").ap()
    wp = nc.dram_tensor("wp", [DH, C], BF16, kind="ExternalInput").ap()
    cos2 = nc.dram_tensor("cos2", [P, N], BF16, kind="ExternalInput").ap()
    sin2 = nc.dram_tensor("sin2", [P, N], BF16, kind="ExternalInput").ap()
    p2t = nc.dram_tensor("p2t", [P, P], BF16, kind="ExternalInput").ap()
    out = nc.dram_tensor("out", [N, C], F32, kind="ExternalOutput").ap()

    def merged(src, rows, blocks, width, off=0):
        # [blocks*128, width] dram slab -> [128, blocks, width] sbuf tile
        return bass.AP(tensor=src.tensor, offset=off,
                       ap=[[rows, P], [P * rows, blocks], [1, width]])

    ctx = ExitStack()
    with ctx:
        consts = ctx.enter_context(tc.tile_pool(name="consts", bufs=1))
        persist = ctx.enter_context(tc.tile_pool(name="persist", bufs=1))

        cos_sb = consts.tile([P, N], BF16, tag="cos")
        sin_sb = consts.tile([P, N], BF16, tag="sin")
        p2t_sb = consts.tile([P, P], BF16, tag="p2t")
        qT = [persist.tile([P, N], BF16, tag=f"qT{t}", name=f"qT{t}")
              for t in range(DHT)]
        kTt = [persist.tile([P, N], BF16, tag=f"kT{t}", name=f"kT{t}")
               for t in range(DHT)]
        v_sb = [persist.tile([P, 8 * 65], BF16, tag=f"v{i}", name=f"v{i}")
                for i in range(NT)]
        # x and wq stay resident: attention-phase filler computes qT c1..c3
        x_sb = [persist.tile([P, CT, 512], BF16, tag=f"x{i}", name=f"x{i}")
                for i in range(NCH)]
        wq_sb = persist.tile([P, CT, 512], BF16, tag="wq", name="wq")
        wp_sb = persist.tile([P, DHT, C], BF16, tag="wp", name="wp")
        aoT = [persist.tile([P, N], BF16, tag=f"aoT{t}", name=f"aoT{t}")
               for t in range(DHT)]

        p1 = ExitStack()
        wpool = p1.enter_context(tc.tile_pool(name="wkv", bufs=1))
        tpool = p1.enter_context(tc.tile_pool(name="p1tmp", bufs=3))
        qk_ps_pool = p1.enter_context(
            tc.tile_pool(name="p1ps", bufs=2, space="PSUM"))
        rot_ps_pool = p1.enter_context(
            tc.tile_pool(name="p1ps2", bufs=2, space="PSUM"))
        v_ps_pool = p1.enter_context(
            tc.tile_pool(name="p1ps3", bufs=2, space="PSUM"))

        wk_sb = wpool.tile([P, CT, 512], BF16, tag="wk", name="wk")
        wv_sb = wpool.tile([P, CT, 512], BF16, tag="wv", name="wv")

        # ---- DMA issue order: first compute needs x0+wk ----
        def load_x(nch):
            for g in range(4):
                sub = bass.AP(tensor=xT.tensor,
                              offset=(g * 2 * P) * N + nch * 512,
                              ap=[[N, P], [P * N, 2], [1, 512]])
                nc.sync.dma_start(x_sb[nch][:, 2 * g:2 * g + 2], sub)
        load_x(0)
        nc.sync.dma_start(wk_sb, merged(wk, DH, CT, DH))
        nc.sync.dma_start(wq_sb, merged(wq, DH, CT, DH))
        load_x(1)
        nc.sync.dma_start(p2t_sb, p2t)
        nc.sync.dma_start(cos_sb, cos2)
        nc.sync.dma_start(sin_sb, sin2)
        load_x(2)
        nc.sync.dma_start(wv_sb, merged(wv, DH, CT, DH))
        load_x(3)
        nc.sync.dma_start(wp_sb, merged(wp, C, DHT, C))
        for i in range(NT):   # softmax-denominator ones column of v
            ones_cols = bass.AP(
                tensor=v_sb[i].tensor, offset=64,
                ap=[list(v_sb[i].ap[0]), [65, 8]])
            nc.vector.memset(ones_cols, 1.0)

        # ---- phase 1 head: k (+rope, rot delayed 1 tile), v, q chunk 0 ----
        rot_pend = []

        def emit_rot():
            raw, dst, nsl = rot_pend.pop(0)
            rot = rot_ps_pool.tile([P, 512], F32, tag="rot_ps")
            nc.tensor.matmul(rot, p2t_sb, raw, start=True, stop=True)
            t1 = tpool.tile([P, 512], F32, tag="t1")
            nc.vector.tensor_mul(t1, raw, cos_sb[:, nsl])
            t2 = tpool.tile([P, 512], F32, tag="t2")
            nc.vector.tensor_mul(t2, rot, sin_sb[:, nsl])
            nc.vector.tensor_add(dst[:, nsl], t1, t2)

        def emit_qk_tile(w_sb, dst, t, nch):
            nsl = slice(nch * 512, (nch + 1) * 512)
            ps = qk_ps_pool.tile([P, 512], F32, tag="qk_ps")
            for kc in range(CT):
                nc.tensor.matmul(
                    ps, w_sb[:, kc, t * P:(t + 1) * P], x_sb[nch][:, kc],
                    start=(kc == 0), stop=(kc == CT - 1))
            raw = tpool.tile([P, 512], BF16, tag="raw")
            nc.scalar.copy(raw, ps)
            rot_pend.append((raw, dst, nsl))
            if len(rot_pend) > 1:
                emit_rot()

        for nch in range(NCH):
            for t in range(DHT):
                emit_qk_tile(wk_sb, kTt[t], t, nch)
        for nch in range(NCH):
            for nt4 in range(4):
                i = nch * 4 + nt4
                ps = v_ps_pool.tile([P, 512], F32, tag="v_ps")
                for kc in range(CT):
                    nc.tensor.matmul(
                        ps, x_sb[nch][:, kc, nt4 * P:(nt4 + 1) * P],
                        wv_sb[:, kc],
                        start=(kc == 0), stop=(kc == CT - 1))
                v_view = bass.AP(
                    tensor=v_sb[i].tensor, offset=0,
                    ap=[list(v_sb[i].ap[0]), [65, 8], [1, 64]])
                nc.scalar.copy(v_view, ps.rearrange("p (h d) -> p h d", h=8))
        for t in range(DHT):
            emit_qk_tile(wq_sb, qT[t], t, 0)
        while rot_pend:
            emit_rot()
        p1.close()

        # ---------------- attention + proj ----------------
        attn_ctx = ExitStack()
        epool = attn_ctx.enter_context(tc.tile_pool(name="epool", bufs=3))
        atmp = attn_ctx.enter_context(tc.tile_pool(name="atmp", bufs=3))
        ftmp = attn_ctx.enter_context(tc.tile_pool(name="ftmp", bufs=2))
        s_ps_pool = attn_ctx.enter_context(
            tc.tile_pool(name="s_ps", bufs=1, space="PSUM"))
        o_ps_pool = attn_ctx.enter_context(
            tc.tile_pool(name="o_ps", bufs=1, space="PSUM"))
        f_ps_pool = attn_ctx.enter_context(
            tc.tile_pool(name="f_ps", bufs=1, space="PSUM"))
        pending = []   # (fn, is_pe_heavy)

        def make_q_pieces(t, nch):
            # qT tile for chunk nch as 1-matmul filler pieces; DVE does the
            # psum->sbuf copy (ScalarE is exp-bound)
            nsl = slice(nch * 512, (nch + 1) * 512)
            box = {}
            pieces = []

            def qmm(kc):
                def fn():
                    if kc == 0:
                        box["ps"] = f_ps_pool.tile([P, 512], F32, tag="fa",
                                                   name="qf")
                    nc.tensor.matmul(
                        box["ps"], wq_sb[:, kc, t * P:(t + 1) * P],
                        x_sb[nch][:, kc],
                        start=(kc == 0), stop=(kc == CT - 1))
                return fn
            for kc in range(CT):
                pieces.append((qmm(kc), True, ("q", nch)))

            def qraw():
                raw = ftmp.tile([P, 512], BF16, tag="fraw")
                box["raw"] = raw
                nc.vector.tensor_copy(raw, box["ps"])

            def qrot():
                rot = f_ps_pool.tile([P, 512], F32, tag="fb", name="qr")
                box["rot"] = rot
                nc.tensor.matmul(rot, p2t_sb, box["raw"], start=True,
                                 stop=True)

            def qt1():
                t1 = ftmp.tile([P, 512], F32, tag="ft1")
                box["t1"] = t1
                nc.vector.tensor_mul(t1, box["raw"], cos_sb[:, nsl])

            def qt2():
                t2 = ftmp.tile([P, 512], F32, tag="ft2")
                nc.vector.tensor_mul(t2, box["rot"], sin_sb[:, nsl])
                nc.vector.tensor_add(qT[t][:, nsl], box["t1"], t2)
            pieces += [(qraw, False, ("q", nch)), (qrot, True, ("q", nch)),
                       (qt1, False, ("q", nch)), (qt2, False, ("q", nch))]
            return pieces

        def make_proj_pieces(nt, fc):
            box = {}
            pieces = []

            def pmm(ct):
                def fn():
                    if ct == 0:
                        box["ps"] = f_ps_pool.tile([P, 512], F32, tag="fa",
                                                   name="pps")
                    nc.tensor.matmul(
                        box["ps"], aoT[ct][:, nt * P:(nt + 1) * P],
                        wp_sb[:, ct, fc * 512:(fc + 1) * 512],
                        start=(ct == 0), stop=(ct == DHT - 1))
                return fn
            for ct in range(DHT):
                pieces.append((pmm(ct), True, ("p",)))

            def pout():
                ob = atmp.tile([P, 512], F32, tag="ob")
                nc.vector.tensor_copy(ob, box["ps"])
                nc.sync.dma_start(
                    out[nt * P:(nt + 1) * P, fc * 512:(fc + 1) * 512], ob)
            pieces.append((pout, False, ("p",)))
            return pieces

        def emit_norm(o_ps_par, hp, qsl, pb_):
            # all DVE/gpsimd: reciprocal straight off the psum denominator
            # row, partition-broadcast, multiply into aoT
            dd = atmp.tile([1, 512], F32, tag="dd")
            nc.vector.tensor_copy(dd, o_ps_par[64:65, :])
            r32 = atmp.tile([1, 512], F32, tag="r32")
            nc.vector.reciprocal_approx_fast(r32, dd)
            rb = atmp.tile([P, 512], F32, tag="rb")
            nc.gpsimd.partition_broadcast(rb[0:64, :], r32, channels=64)
            nc.vector.tensor_mul(
                aoT[hp][pb_:pb_ + 64, qsl], o_ps_par[0:64, :],
                rb[0:64, :])

        def pop_filler(pe_budget=1):
            pops = 0
            while pending and pe_budget > 0 and pops < 5:
                fn, heavy, _ = pending.pop(0)
                fn()
                pops += 1
                if heavy:
                    pe_budget -= 1

        def drain_q_chunk(nch):
            # safety: everything that writes qT chunk nch must be emitted
            # before the first scores read of that chunk
            while any(t == ("q", nch) for _, _, t in pending):
                fn, _, _ = pending.pop(0)
                fn()

        # queue qT tiles for chunks 1..3 ahead of the proj work
        for nch in range(1, NCH):
            for t in range(DHT):
                pending.extend(make_q_pieces(t, nch))

        for qc in range(NCH):
            qsl = slice(qc * 512, (qc + 1) * 512)
            if qc > 0:
                drain_q_chunk(qc)
            for hp in range(4):      # head pairs (even@part0-63, odd@64-127)
                o_ps = {}
                for par in range(2):  # par=0: even head, par=1: odd head
                    o_ps[par] = o_ps_pool.tile([P, 512], F32,
                                               tag=f"o{par}", name=f"o{par}")

                def emit_av(ki, e):
                    for par in range(2):
                        h = hp * 2 + par
                        # rows 0-63: attn@v; row 64: softmax denominator
                        nc.tensor.matmul(
                            o_ps[par][0:65, :],
                            v_sb[ki][:, h * 65:(h + 1) * 65],
                            e[:, par],
                            start=(ki == 0), stop=(ki == KT - 1))

                prev = None
                for ki in range(KT):
                    ksl = slice(ki * P, (ki + 1) * P)
                    s_ps = s_ps_pool.tile([P, 2, 512], F32,
                                          tag=f"s{ki % 2}", name=f"s{ki % 2}")
                    for par in range(2):
                        pb_ = par * 64
                        nc.tensor.matmul(
                            s_ps[:, par],
                            kTt[hp][pb_:pb_ + 64, ksl],
                            qT[hp][pb_:pb_ + 64, qsl],
                            start=True, stop=True,
                            tile_position=(pb_, 0))
                    if prev is not None:
                        emit_av(*prev)
                    pop_filler(pe_budget=2 if qc == NCH - 1 else 1)
                    e = epool.tile([P, 2, 512], BF16, tag="e", name="e")
                    nc.scalar.activation(e, s_ps, Exp,
                                         scale=float(D) ** -0.5)
                    prev = (ki, e)
                emit_av(*prev)
                for par in range(2):
                    emit_norm(o_ps[par], hp, qsl, par * 64)
            for nt in range(qc * 4, (qc + 1) * 4):
                for fc in range(2):
                    pending.extend(make_proj_pieces(nt, fc))
        for fn, _, _ in pending:
            fn()
        attn_ctx.close()


def build_nc():
    if "nc" in _CACHE:
        return _CACHE["nc"]
    import concourse.bass as bass
    import concourse.tile as tile
    from concourse import bacc, mybir

    nc = bacc.Bacc("TRN2", target_bir_lowering=False, debug=False,
                   enable_asserts=False, num_devices=NCORES)
    with tile.TileContext(nc) as tc:
        _emit(nc, tc, mybir, bass, tile)
    nc.compile()
    _CACHE["nc"] = nc
    return nc


def make_in_maps(x, rope_cos, rope_sin, w_qkv, w_proj):
    import ml_dtypes
    BF = ml_dtypes.bfloat16

    x = np.asarray(x, dtype=np.float32)
    rope_cos = np.asarray(rope_cos, dtype=np.float32)
    rope_sin = np.asarray(rope_sin, dtype=np.float32)
    w_qkv = np.asarray(w_qkv, dtype=np.float32)
    w_proj = np.asarray(w_proj, dtype=np.float32)

    cosT = np.ascontiguousarray(rope_cos.T)           # [64, N]
    cos2 = np.vstack([cosT, cosT]).astype(BF)         # [128, N]
    sinT = np.ascontiguousarray(rope_sin.T)
    sin2 = np.vstack([sinT, sinT]).astype(BF)

    # signed half-rotation permutation: rot(q) = P2 @ q (per 64-block)
    R = np.zeros((D, D), dtype=np.float32)
    half = D // 2
    R[np.arange(half), np.arange(half) + half] = -1.0
    R[np.arange(half) + half, np.arange(half)] = 1.0
    P2 = np.zeros((P, P), dtype=np.float32)
    P2[:D, :D] = R
    P2[D:, D:] = R
    p2t = np.ascontiguousarray(P2.T).astype(BF)

    xTs = [np.ascontiguousarray(x[b].T).astype(BF) for b in range(B)]

    in_maps = []
    for core in range(NCORES):
        b = core // 2
        hg = core % 2
        in_maps.append({
            "xT": xTs[b],
            "wq": np.ascontiguousarray(
                w_qkv[hg * DH:(hg + 1) * DH, :].T).astype(BF),
            "wk": np.ascontiguousarray(
                w_qkv[C + hg * DH:C + (hg + 1) * DH, :].T).astype(BF),
            "wv": np.ascontiguousarray(
                w_qkv[2 * C + hg * DH:2 * C + (hg + 1) * DH, :].T).astype(BF),
            "wp": np.ascontiguousarray(
                w_proj[:, hg * DH:(hg + 1) * DH].T).astype(BF),
            "cos2": cos2,
            "sin2": sin2,
            "p2t": p2t,
        })
    return in_maps


def kernel(x, rope_cos, rope_sin, w_qkv, w_proj, b_proj, trace=False):
    from concourse.bass_utils import run_bass_kernel_spmd

    nc = build_nc()
    in_maps = make_in_maps(x, rope_cos, rope_sin, w_qkv, w_proj)
    res = run_bass_kernel_spmd(nc, in_maps, core_ids=list(range(NCORES)),
                               trace=trace)
    b_proj = np.asarray(b_proj, dtype=np.float32)
    final = np.empty((B, N, C), dtype=np.float32)
    for b in range(B):
        final[b] = res.results[2 * b]["out"] + res.results[2 * b + 1]["out"] \
            + b_proj
    if trace:
        kernel.last_exec_time_ns = res.exec_time_ns
        kernel.last_results = res
    return final


# revision 22
# speedup vs baseline: 1.1321x; 1.1321x over previous
"""Fused multi-head attention (qkv + RoPE + softmax + proj) for TRN2, 8 cores.

Sharding: core c -> batch b=c//2, head group hg=c%2 (8 of 16 heads).
Data-parallel over B (4), 2-way tensor-parallel over heads.
Host unshard: out[b] = partial[2b] + partial[2b+1] + b_proj.

v4: all matmul operands bf16 (1 cyc/col @2.4GHz).  The attention loop is
gated by the ScalarE softmax exp at ~(1024+340)/1.2GHz ~= 1.12us per key
tile; everything else is paced to hide under it:
  head:  k^T (+rope; the rot matmul is delayed one tile so the PE never
         waits on the psum->sbuf copy), v, q^T chunk 0.
  loop:  scores pair (tile_position quadrants) -> exp -> attn@v pair, plus
         a filler queue popped once per ki at 1-matmul granularity:
         remaining q^T tiles, proj tiles, and the block-boundary softmax
         normalization (reciprocal on DVE, partition-broadcast via a
         [1,64] ones matmul, psum x psum multiply into aoT on DVE).
PSUM budget: scores 2x2 banks, attn@v accumulators 2 (single-buffered),
filler accum 1, rot/broadcast 1.
"""

import sys

if "/opt/trn_rl_repo" not in sys.path:
    sys.path.insert(0, "/opt/trn_rl_repo")

import numpy as np
from contextlib import ExitStack

B, N, C, H, D = 4, 2048, 1024, 16, 64
NCORES = 8
P = 128
DH = 512          # per-core head channels (8 heads x 64)
CT = C // P       # 8 contraction tiles for qkv
DHT = DH // P     # 4 partition tiles of qT/kT/aoT
NT = N // P       # 16 n tiles
NCH = N // 512    # 4 n chunks of 512
KT = N // P       # 16 key tiles

_CACHE = {}


def _emit(nc, tc, mybir, bass, tile):
    F32 = mybir.dt.float32
    BF16 = mybir.dt.bfloat16
    FP16 = mybir.dt.float16
    Exp = mybir.ActivationFunctionType.Exp

    xT = nc.dram_tensor("xT", [C, N], BF16, kind="ExternalInput").ap()
    wq = nc.dram_tensor("wq", [C, DH], BF16, kind="ExternalInput").ap()
    wk = nc.dram_tensor("wk", [C, DH], BF16, kind="ExternalInput").ap()
    wv = nc.dram_tensor("wv", [C, DH], BF16, kind="ExternalInput").ap()
    wp = nc.dram_tensor("wp", [DH, C], BF16, kind="ExternalInput").ap()
    cos2 = nc.dram_tensor("cos2", [P, N], BF16, kind="ExternalInput").ap()
    sin2 = nc.dram_tensor("sin2", [P, N], BF16, kind="ExternalInput").ap()
    p2t = nc.dram_tensor("p2t", [P, P], BF16, kind="ExternalInput").ap()
    out = nc.dram_tensor("out", [N, C], F32, kind="ExternalOutput").ap()

    def merged(src, rows, blocks, width, off=0):
        # [blocks*128, width] dram slab -> [128, blocks, width] sbuf tile
        return bass.AP(tensor=src.tensor, offset=off,
                       ap=[[rows, P], [P * rows, blocks], [1, width]])

    ctx = ExitStack()
    with ctx:
        consts = ctx.enter_context(tc.tile_pool(name="consts", bufs=1))
        persist = ctx.enter_context(tc.tile_pool(name="persist", bufs=1))

        cos_sb = consts.tile([P, N], BF16, tag="cos")
        sin_sb = consts.tile([P, N], BF16, tag="sin")
        p2t_sb = consts.tile([P, P], BF16, tag="p2t")
        qT = [persist.tile([P, N], BF16, tag=f"qT{t}", name=f"qT{t}")
              for t in range(DHT)]
        kTt = [persist.tile([P, N], BF16, tag=f"kT{t}", name=f"kT{t}")
               for t in range(DHT)]
        v_sb = [persist.tile([P, 8 * 65], BF16, tag=f"v{i}", name=f"v{i}")
                for i in range(NT)]
        # x and wq stay resident: attention-phase filler computes qT c1..c3
        x_sb = [persist.tile([P, CT, 512], BF16, tag=f"x{i}", name=f"x{i}")
                for i in range(NCH)]
        wq_sb = persist.tile([P, CT, 512], BF16, tag="wq", name="wq")
        wp_sb = persist.tile([P, DHT, C], BF16, tag="wp", name="wp")
        aoT = [persist.tile([P, N], BF16, tag=f"aoT{t}", name=f"aoT{t}")
               for t in range(DHT)]

        p1 = ExitStack()
        wpool = p1.enter_context(tc.tile_pool(name="wkv", bufs=1))
        tpool = p1.enter_context(tc.tile_pool(name="p1tmp", bufs=3))
        qk_ps_pool = p1.enter_context(
            tc.tile_pool(name="p1ps", bufs=2, space="PSUM"))
        rot_ps_pool = p1.enter_context(
            tc.tile_pool(name="p1ps2", bufs=2, space="PSUM"))
        v_ps_pool = p1.enter_context(
            tc.tile_pool(name="p1ps3", bufs=2, space="PSUM"))

        wk_sb = wpool.tile([P, CT, 512], BF16, tag="wk", name="wk")
        wv_sb = wpool.tile([P, CT, 512], BF16, tag="wv", name="wv")

        # ---- DMA issue order: first compute needs x0+wk ----
        def load_x(nch):
            for g in range(4):
                sub = bass.AP(tensor=xT.tensor,
                              offset=(g * 2 * P) * N + nch * 512,
                              ap=[[N, P], [P * N, 2], [1, 512]])
                nc.sync.dma_start(x_sb[nch][:, 2 * g:2 * g + 2], sub)
        load_x(0)
        nc.sync.dma_start(wk_sb, merged(wk, DH, CT, DH))
        nc.sync.dma_start(wq_sb, merged(wq, DH, CT, DH))
        load_x(1)
        nc.sync.dma_start(p2t_sb, p2t)
        nc.sync.dma_start(cos_sb, cos2)
        nc.sync.dma_start(sin_sb, sin2)
        load_x(2)
        nc.sync.dma_start(wv_sb, merged(wv, DH, CT, DH))
        load_x(3)
        nc.sync.dma_start(wp_sb, merged(wp, C, DHT, C))
        for i in range(NT):   # softmax-denominator ones column of v
            ones_cols = bass.AP(
                tensor=v_sb[i].tensor, offset=64,
                ap=[list(v_sb[i].ap[0]), [65, 8]])
            nc.vector.memset(ones_cols, 1.0)

        # ---- phase 1 head: k (+rope, rot delayed 1 tile), v, q chunk 0 ----
        rot_pend = []

        def emit_rot():
            raw, dst, nsl = rot_pend.pop(0)
            rot = rot_ps_pool.tile([P, 512], F32, tag="rot_ps")
            nc.tensor.matmul(rot, p2t_sb, raw, start=True, stop=True)
            t1 = tpool.tile([P, 512], F32, tag="t1")
            nc.vector.tensor_mul(t1, raw, cos_sb[:, nsl])
            t2 = tpool.tile([P, 512], F32, tag="t2")
            nc.vector.tensor_mul(t2, rot, sin_sb[:, nsl])
            nc.vector.tensor_add(dst[:, nsl], t1, t2)

        def emit_qk_tile(w_sb, dst, t, nch):
            nsl = slice(nch * 512, (nch + 1) * 512)
            ps = qk_ps_pool.tile([P, 512], F32, tag="qk_ps")
            for kc in range(CT):
                nc.tensor.matmul(
                    ps, w_sb[:, kc, t * P:(t + 1) * P], x_sb[nch][:, kc],
                    start=(kc == 0), stop=(kc == CT - 1))
            raw = tpool.tile([P, 512], BF16, tag="raw")
            nc.scalar.copy(raw, ps)
            rot_pend.append((raw, dst, nsl))
            if len(rot_pend) > 1:
                emit_rot()

        for nch in range(NCH):
            for t in range(DHT):
                emit_qk_tile(wk_sb, kTt[t], t, nch)
        for nch in range(NCH):
            for nt4 in range(4):
                i = nch * 4 + nt4
                ps = v_ps_pool.tile([P, 512], F32, tag="v_ps")
                for kc in range(CT):
                    nc.tensor.matmul(
                        ps, x_sb[nch][:, kc, nt4 * P:(nt4 + 1) * P],
                        wv_sb[:, kc],
                        start=(kc == 0), stop=(kc == CT - 1))
                v_view = bass.AP(
                    tensor=v_sb[i].tensor, offset=0,
                    ap=[list(v_sb[i].ap[0]), [65, 8], [1, 64]])
                nc.scalar.copy(v_view, ps.rearrange("p (h d) -> p h d", h=8))
        for nch in range(NCH):
            for t in range(DHT):
                emit_qk_tile(wq_sb, qT[t], t, nch)
        while rot_pend:
            emit_rot()
        p1.close()

        # ---------------- attention + proj ----------------
        attn_ctx = ExitStack()
        epool = attn_ctx.enter_context(tc.tile_pool(name="epool", bufs=3))
        atmp = attn_ctx.enter_context(tc.tile_pool(name="atmp", bufs=3))
        s_ps_pool = attn_ctx.enter_context(
            tc.tile_pool(name="s_ps", bufs=1, space="PSUM"))
        o_ps_pool = attn_ctx.enter_context(
            tc.tile_pool(name="o_ps", bufs=2, space="PSUM"))
        pending = []   # filler pieces (closures)

        def make_proj_pieces(nt, fc):
            # one output tile's proj as two 2-matmul PE filler pieces; the
            # psum tile is created by piece A and finished by piece B
            box = {}

            def pa():
                ps = o_ps_pool.tile([P, 512], F32, tag=f"o{nt % 2}",
                                    name="pps")
                box["ps"] = ps
                for ct in range(2):
                    nc.tensor.matmul(
                        ps, aoT[ct][:, nt * P:(nt + 1) * P],
                        wp_sb[:, ct, fc * 512:(fc + 1) * 512],
                        start=(ct == 0), stop=False)

            def pb():
                ps = box["ps"]
                for ct in range(2, DHT):
                    nc.tensor.matmul(
                        ps, aoT[ct][:, nt * P:(nt + 1) * P],
                        wp_sb[:, ct, fc * 512:(fc + 1) * 512],
                        start=False, stop=(ct == DHT - 1))
                ob = atmp.tile([P, 512], F32, tag="ob")
                nc.vector.tensor_copy(ob, ps)
                nc.sync.dma_start(
                    out[nt * P:(nt + 1) * P, fc * 512:(fc + 1) * 512], ob)
            return [pa, pb]

        def emit_norm(o_ps_par, hp, qsl, pb_):
            dd = atmp.tile([1, 512], F32, tag="dd")
            nc.vector.tensor_copy(dd, o_ps_par[64:65, :])
            r32 = atmp.tile([1, 512], F32, tag="r32")
            nc.vector.reciprocal_approx_fast(r32, dd)
            rb = atmp.tile([P, 512], F32, tag="rb")
            nc.gpsimd.partition_broadcast(rb[0:64, :], r32, channels=64)
            nc.vector.tensor_mul(
                aoT[hp][pb_:pb_ + 64, qsl], o_ps_par[0:64, :],
                rb[0:64, :])

        for qc in range(NCH):
            qsl = slice(qc * 512, (qc + 1) * 512)
            for hp in range(4):      # head pairs (even@part0-63, odd@64-127)
                o_ps = {}
                for par in range(2):  # par=0: even head, par=1: odd head
                    o_ps[par] = o_ps_pool.tile([P, 512], F32,
                                               tag=f"o{par}", name=f"o{par}")

                def emit_av(ki, e):
                    for par in range(2):
                        h = hp * 2 + par
                        # rows 0-63: attn@v; row 64: softmax denominator
                        nc.tensor.matmul(
                            o_ps[par][0:65, :],
                            v_sb[ki][:, h * 65:(h + 1) * 65],
                            e[:, par],
                            start=(ki == 0), stop=(ki == KT - 1))

                prev = None
                for ki in range(KT):
                    ksl = slice(ki * P, (ki + 1) * P)
                    s_ps = s_ps_pool.tile([P, 2, 512], F32,
                                          tag=f"s{ki % 2}", name=f"s{ki % 2}")
                    for par in range(2):
                        pb_ = par * 64
                        nc.tensor.matmul(
                            s_ps[:, par],
                            kTt[hp][pb_:pb_ + 64, ksl],
                            qT[hp][pb_:pb_ + 64, qsl],
                            start=True, stop=True,
                            tile_position=(pb_, 0))
                    if prev is not None:
                        emit_av(*prev)
                    if pending and ((ki >= 2 and ki % 3 == 2)
                                    or (qc == NCH - 1 and ki % 2 == 1)):
                        pending.pop(0)()
                    e = epool.tile([P, 2, 512], BF16, tag="e", name="e")
                    nc.scalar.activation(e, s_ps, Exp,
                                         scale=float(D) ** -0.5)
                    prev = (ki, e)
                emit_av(*prev)
                for par in range(2):
                    emit_norm(o_ps[par], hp, qsl, par * 64)
            for nt in range(qc * 4, (qc + 1) * 4):
                for fc in range(2):
                    pending.extend(make_proj_pieces(nt, fc))
        for fn in pending:
            fn()
        attn_ctx.close()


def build_nc():
    if "nc" in _CACHE:
        return _CACHE["nc"]
    import concourse.bass as bass
    import concourse.tile as tile
    from concourse import bacc, mybir

    nc = bacc.Bacc("TRN2", target_bir_lowering=False, debug=False,
                   enable_asserts=False, num_devices=NCORES)
    with tile.TileContext(nc) as tc:
        _emit(nc, tc, mybir, bass, tile)
    nc.compile()
    _CACHE["nc"] = nc
    return nc


def make_in_maps(x, rope_cos, rope_sin, w_qkv, w_proj):
    import ml_dtypes
    BF = ml_dtypes.bfloat16

    x = np.asarray(x, dtype=np.float32)
    rope_cos = np.asarray(rope_cos, dtype=np.float32)
    rope_sin = np.asarray(rope_sin, dtype=np.float32)
    w_qkv = np.asarray(w_qkv, dtype=np.float32)
    w_proj = np.asarray(w_proj, dtype=np.float32)

    cosT = np.ascontiguousarray(rope_cos.T)           # [64, N]
    cos2 = np.vstack([cosT, cosT]).astype(BF)         # [128, N]
    sinT = np.ascontiguousarray(rope_sin.T)
    sin2 = np.vstack([sinT, sinT]).astype(BF)

    # signed half-rotation permutation: rot(q) = P2 @ q (per 64-block)
    R = np.zeros((D, D), dtype=np.float32)
    half = D // 2
    R[np.arange(half), np.arange(half) + half] = -1.0
    R[np.arange(half) + half, np.arange(half)] = 1.0
    P2 = np.zeros((P, P), dtype=np.float32)
    P2[:D, :D] = R
    P2[D:, D:] = R
    p2t = np.ascontiguousarray(P2.T).astype(BF)

    xTs = [np.ascontiguousarray(x[b].T).astype(BF) for b in range(B)]

    in_maps = []
    for core in range(NCORES):
        b = core // 2
        hg = core % 2
        in_maps.append({
            "xT": xTs[b],
            "wq": np.ascontiguousarray(
                w_qkv[hg * DH:(hg + 1) * DH, :].T).astype(BF),
            "wk": np.ascontiguousarray(
                w_qkv[C + hg * DH:C + (hg + 1) * DH, :].T).astype(BF),
            "wv": np.ascontiguousarray(
                w_qkv[2 * C + hg * DH:2 * C + (hg + 1) * DH, :].T).astype(BF),
            "wp": np.ascontiguousarray(
                w_proj[:, hg * DH:(hg + 1) * DH].T).astype(BF),
            "cos2": cos2,
            "sin2": sin2,
            "p2t": p2t,
        })
    return in_maps


def kernel(x, rope_cos, rope_sin, w_qkv, w_proj, b_proj, trace=False):
    from concourse.bass_utils import run_bass_kernel_spmd

    nc = build_nc()
    in_maps = make_in_maps(x, rope_cos, rope_sin, w_qkv, w_proj)
    res = run_bass_kernel_spmd(nc, in_maps, core_ids=list(range(NCORES)),
                               trace=trace)
    b_proj = np.asarray(b_proj, dtype=np.float32)
    final = np.empty((B, N, C), dtype=np.float32)
    for b in range(B):
        final[b] = res.results[2 * b]["out"] + res.results[2 * b + 1]["out"] \
            + b_proj
    if trace:
        kernel.last_exec_time_ns = res.exec_time_ns
        kernel.last_results = res
    return final


# revision 23
# speedup vs baseline: 1.2880x; 1.1377x over previous
"""Fused multi-head attention (qkv + RoPE + softmax + proj) for TRN2, 8 cores.

Sharding: core c -> batch b=c//2, head group hg=c%2 (8 of 16 heads).
Data-parallel over B (4), 2-way tensor-parallel over heads.
Host unshard: out[b] = partial[2b] + partial[2b+1] + b_proj.

v4: all matmul operands bf16 (1 cyc/col @2.4GHz).  The attention loop is
gated by the ScalarE softmax exp at ~(1024+340)/1.2GHz ~= 1.12us per key
tile; everything else is paced to hide under it:
  head:  k^T (+rope; the rot matmul is delayed one tile so the PE never
         waits on the psum->sbuf copy), v, q^T chunk 0.
  loop:  scores pair (tile_position quadrants) -> exp -> attn@v pair, plus
         a filler queue popped once per ki at 1-matmul granularity:
         remaining q^T tiles, proj tiles, and the block-boundary softmax
         normalization (reciprocal on DVE, partition-broadcast via a
         [1,64] ones matmul, psum x psum multiply into aoT on DVE).
PSUM budget: scores 2x2 banks, attn@v accumulators 2 (single-buffered),
filler accum 1, rot/broadcast 1.
"""

import sys

if "/opt/trn_rl_repo" not in sys.path:
    sys.path.insert(0, "/opt/trn_rl_repo")

import numpy as np
from contextlib import ExitStack

B, N, C, H, D = 4, 2048, 1024, 16, 64
NCORES = 8
P = 128
DH = 512          # per-core head channels (8 heads x 64)
CT = C // P       # 8 contraction tiles for qkv
DHT = DH // P     # 4 partition tiles of qT/kT/aoT
NT = N // P       # 16 n tiles
NCH = N // 512    # 4 n chunks of 512
KT = N // P       # 16 key tiles

_CACHE = {}


def _emit(nc, tc, mybir, bass, tile):
    F32 = mybir.dt.float32
    BF16 = mybir.dt.bfloat16
    FP16 = mybir.dt.float16
    Exp = mybir.ActivationFunctionType.Exp

    xT = nc.dram_tensor("xT", [C, N], BF16, kind="ExternalInput").ap()
    wq = nc.dram_tensor("wq", [C, DH], BF16, kind="ExternalInput").ap()
    wk = nc.dram_tensor("wk", [C, DH], BF16, kind="ExternalInput").ap()
    wv = nc.dram_tensor("wv", [C, DH], BF16, kind="ExternalInput").ap()
    wp = nc.dram_tensor("wp", [DH, C], BF16, kind="ExternalInput").ap()
    cos2 = nc.dram_tensor("cos2", [P, N], BF16, kind="ExternalInput").ap()
    sin2 = nc.dram_tensor("sin2", [P, N], BF16, kind="ExternalInput").ap()
    p2t = nc.dram_tensor("p2t", [P, P], BF16, kind="ExternalInput").ap()
    out = nc.dram_tensor("out", [N, C], F32, kind="ExternalOutput").ap()

    def merged(src, rows, blocks, width, off=0):
        # [blocks*128, width] dram slab -> [128, blocks, width] sbuf tile
        return bass.AP(tensor=src.tensor, offset=off,
                       ap=[[rows, P], [P * rows, blocks], [1, width]])

    ctx = ExitStack()
    with ctx:
        consts = ctx.enter_context(tc.tile_pool(name="consts", bufs=1))
        persist = ctx.enter_context(tc.tile_pool(name="persist", bufs=1))

        cos_sb = consts.tile([P, N], BF16, tag="cos")
        sin_sb = consts.tile([P, N], BF16, tag="sin")
        p2t_sb = consts.tile([P, P], BF16, tag="p2t")
        qT = [persist.tile([P, N], BF16, tag=f"qT{t}", name=f"qT{t}")
              for t in range(DHT)]
        kTt = [persist.tile([P, N], BF16, tag=f"kT{t}", name=f"kT{t}")
               for t in range(DHT)]
        v_sb = [persist.tile([P, 8 * 65], BF16, tag=f"v{i}", name=f"v{i}")
                for i in range(NT)]
        # x and wq stay resident: attention-phase filler computes qT c1..c3
        x_sb = [persist.tile([P, CT, 512], BF16, tag=f"x{i}", name=f"x{i}")
                for i in range(NCH)]
        wq_sb = persist.tile([P, CT, 512], BF16, tag="wq", name="wq")
        wp_sb = persist.tile([P, DHT, C], BF16, tag="wp", name="wp")
        aoT = [persist.tile([P, N], BF16, tag=f"aoT{t}", name=f"aoT{t}")
               for t in range(DHT)]

        p1 = ExitStack()
        wpool = p1.enter_context(tc.tile_pool(name="wkv", bufs=1))
        tpool = p1.enter_context(tc.tile_pool(name="p1tmp", bufs=3))
        qk_ps_pool = p1.enter_context(
            tc.tile_pool(name="p1ps", bufs=2, space="PSUM"))
        rot_ps_pool = p1.enter_context(
            tc.tile_pool(name="p1ps2", bufs=2, space="PSUM"))
        v_ps_pool = p1.enter_context(
            tc.tile_pool(name="p1ps3", bufs=2, space="PSUM"))

        wk_sb = wpool.tile([P, CT, 512], BF16, tag="wk", name="wk")
        wv_sb = wpool.tile([P, CT, 512], BF16, tag="wv", name="wv")

        # ---- DMA issue order: first compute needs x0+wk ----
        def load_x(nch):
            for g in range(4):
                sub = bass.AP(tensor=xT.tensor,
                              offset=(g * 2 * P) * N + nch * 512,
                              ap=[[N, P], [P * N, 2], [1, 512]])
                nc.sync.dma_start(x_sb[nch][:, 2 * g:2 * g + 2], sub)
        load_x(0)
        nc.sync.dma_start(wk_sb, merged(wk, DH, CT, DH))
        nc.sync.dma_start(wq_sb, merged(wq, DH, CT, DH))
        load_x(1)
        nc.sync.dma_start(p2t_sb, p2t)
        nc.sync.dma_start(cos_sb, cos2)
        nc.sync.dma_start(sin_sb, sin2)
        load_x(2)
        nc.sync.dma_start(wv_sb, merged(wv, DH, CT, DH))
        load_x(3)
        nc.sync.dma_start(wp_sb, merged(wp, C, DHT, C))
        for i in range(NT):   # softmax-denominator ones column of v
            ones_cols = bass.AP(
                tensor=v_sb[i].tensor, offset=64,
                ap=[list(v_sb[i].ap[0]), [65, 8]])
            nc.vector.memset(ones_cols, 1.0)

        # ---- phase 1 head: k (+rope, rot delayed 1 tile), v, q chunk 0 ----
        rot_pend = []

        def emit_rot():
            raw, dst, nsl = rot_pend.pop(0)
            rot = rot_ps_pool.tile([P, 512], F32, tag="rot_ps")
            nc.tensor.matmul(rot, p2t_sb, raw, start=True, stop=True)
            t1 = tpool.tile([P, 512], F32, tag="t1")
            nc.vector.tensor_mul(t1, raw, cos_sb[:, nsl])
            t2 = tpool.tile([P, 512], F32, tag="t2")
            nc.vector.tensor_mul(t2, rot, sin_sb[:, nsl])
            nc.vector.tensor_add(dst[:, nsl], t1, t2)

        def emit_qk_tile(w_sb, dst, t, nch):
            nsl = slice(nch * 512, (nch + 1) * 512)
            ps = qk_ps_pool.tile([P, 512], F32, tag="qk_ps")
            for kc in range(CT):
                nc.tensor.matmul(
                    ps, w_sb[:, kc, t * P:(t + 1) * P], x_sb[nch][:, kc],
                    start=(kc == 0), stop=(kc == CT - 1))
            raw = tpool.tile([P, 512], BF16, tag="raw")
            nc.scalar.copy(raw, ps)
            rot_pend.append((raw, dst, nsl))
            if len(rot_pend) > 1:
                emit_rot()

        for nch in range(NCH):
            for t in range(DHT):
                emit_qk_tile(wk_sb, kTt[t], t, nch)
        for nch in range(NCH):
            for nt4 in range(4):
                i = nch * 4 + nt4
                ps = v_ps_pool.tile([P, 512], F32, tag="v_ps")
                for kc in range(CT):
                    nc.tensor.matmul(
                        ps, x_sb[nch][:, kc, nt4 * P:(nt4 + 1) * P],
                        wv_sb[:, kc],
                        start=(kc == 0), stop=(kc == CT - 1))
                v_view = bass.AP(
                    tensor=v_sb[i].tensor, offset=0,
                    ap=[list(v_sb[i].ap[0]), [65, 8], [1, 64]])
                nc.scalar.copy(v_view, ps.rearrange("p (h d) -> p h d", h=8))
        for nch in range(NCH):
            for t in range(DHT):
                emit_qk_tile(wq_sb, qT[t], t, nch)
        while rot_pend:
            emit_rot()
        p1.close()

        # ---------------- attention + proj ----------------
        attn_ctx = ExitStack()
        epool = attn_ctx.enter_context(tc.tile_pool(name="epool2", bufs=3))
        atmp = attn_ctx.enter_context(tc.tile_pool(name="atmp", bufs=3))
        s_ps_pool = attn_ctx.enter_context(
            tc.tile_pool(name="s_ps", bufs=1, space="PSUM"))
        o_ps_pool = attn_ctx.enter_context(
            tc.tile_pool(name="o_ps", bufs=2, space="PSUM"))
        pending = []   # filler pieces (closures)

        def make_proj_pieces(nt, fc):
            # one output tile's proj as two 2-matmul PE filler pieces; the
            # psum tile is created by piece A and finished by piece B
            box = {}

            def pa():
                ps = o_ps_pool.tile([P, 512], F32, tag=f"o{nt % 2}",
                                    name="pps")
                box["ps"] = ps
                for ct in range(2):
                    nc.tensor.matmul(
                        ps, aoT[ct][:, nt * P:(nt + 1) * P],
                        wp_sb[:, ct, fc * 512:(fc + 1) * 512],
                        start=(ct == 0), stop=False)

            def pb():
                ps = box["ps"]
                for ct in range(2, DHT):
                    nc.tensor.matmul(
                        ps, aoT[ct][:, nt * P:(nt + 1) * P],
                        wp_sb[:, ct, fc * 512:(fc + 1) * 512],
                        start=False, stop=(ct == DHT - 1))
                ob = atmp.tile([P, 512], F32, tag="ob")
                nc.vector.tensor_copy(ob, ps)
                nc.sync.dma_start(
                    out[nt * P:(nt + 1) * P, fc * 512:(fc + 1) * 512], ob)
            return [pa, pb]

        def emit_norm(o_ps_par, hp, qsl, pb_):
            dd = atmp.tile([1, 512], F32, tag="dd")
            nc.vector.tensor_copy(dd, o_ps_par[64:65, :])
            r32 = atmp.tile([1, 512], F32, tag="r32")
            nc.vector.reciprocal_approx_fast(r32, dd)
            rb = atmp.tile([P, 512], F32, tag="rb")
            nc.gpsimd.partition_broadcast(rb[0:64, :], r32, channels=64)
            nc.vector.tensor_mul(
                aoT[hp][pb_:pb_ + 64, qsl], o_ps_par[0:64, :],
                rb[0:64, :])

        for qc in range(NCH):
            qsl = slice(qc * 512, (qc + 1) * 512)
            for hp in range(4):      # head pairs (even@part0-63, odd@64-127)
                o_ps = {}
                for par in range(2):  # par=0: even head, par=1: odd head
                    o_ps[par] = o_ps_pool.tile([P, 512], F32,
                                               tag=f"o{par}", name=f"o{par}")

                def emit_av(ki, e):
                    for par in range(2):
                        h = hp * 2 + par
                        # rows 0-63: attn@v; row 64: softmax denominator
                        nc.tensor.matmul(
                            o_ps[par][0:65, :],
                            v_sb[ki][:, h * 65:(h + 1) * 65],
                            e[:, par],
                            start=(ki == 0), stop=(ki == KT - 1))

                prev = None
                for ki in range(KT):
                    ksl = slice(ki * P, (ki + 1) * P)
                    s_ps = s_ps_pool.tile([P, 2, 512], F32,
                                          tag=f"s{ki % 2}", name=f"s{ki % 2}")
                    for par in range(2):
                        pb_ = par * 64
                        nc.tensor.matmul(
                            s_ps[:, par],
                            kTt[hp][pb_:pb_ + 64, ksl],
                            qT[hp][pb_:pb_ + 64, qsl],
                            start=True, stop=True,
                            tile_position=(pb_, 0))
                    if prev is not None:
                        emit_av(*prev)
                    if pending and ((ki >= 2 and ki % 3 == 2)
                                    or (qc == NCH - 1 and ki % 2 == 1)):
                        pending.pop(0)()
                    e = epool.tile([P, 2, 512], BF16, tag="e", name="e")
                    nc.scalar.activation(e, s_ps, Exp,
                                         scale=float(D) ** -0.5)
                    prev = (ki, e)
                emit_av(*prev)
                for par in range(2):
                    emit_norm(o_ps[par], hp, qsl, par * 64)
            for nt in range(qc * 4, (qc + 1) * 4):
                for fc in range(2):
                    pending.extend(make_proj_pieces(nt, fc))
        for fn in pending:
            fn()
        attn_ctx.close()


def build_nc():
    if "nc" in _CACHE:
        return _CACHE["nc"]
    import concourse.bass as bass
    import concourse.tile as tile
    from concourse import bacc, mybir

    nc = bacc.Bacc("TRN2", target_bir_lowering=False, debug=False,
                   enable_asserts=False, num_devices=NCORES)
    with tile.TileContext(nc) as tc:
        _emit(nc, tc, mybir, bass, tile)
    nc.compile()
    _CACHE["nc"] = nc
    return nc


def make_in_maps(x, rope_cos, rope_sin, w_qkv, w_proj):
    import ml_dtypes
    BF = ml_dtypes.bfloat16

    x = np.asarray(x, dtype=np.float32)
    rope_cos = np.asarray(rope_cos, dtype=np.float32)
    rope_sin = np.asarray(rope_sin, dtype=np.float32)
    w_qkv = np.asarray(w_qkv, dtype=np.float32)
    w_proj = np.asarray(w_proj, dtype=np.float32)

    cosT = np.ascontiguousarray(rope_cos.T)           # [64, N]
    cos2 = np.vstack([cosT, cosT]).astype(BF)         # [128, N]
    sinT = np.ascontiguousarray(rope_sin.T)
    sin2 = np.vstack([sinT, sinT]).astype(BF)

    # signed half-rotation permutation: rot(q) = P2 @ q (per 64-block)
    R = np.zeros((D, D), dtype=np.float32)
    half = D // 2
    R[np.arange(half), np.arange(half) + half] = -1.0
    R[np.arange(half) + half, np.arange(half)] = 1.0
    P2 = np.zeros((P, P), dtype=np.float32)
    P2[:D, :D] = R
    P2[D:, D:] = R
    p2t = np.ascontiguousarray(P2.T).astype(BF)

    xTs = [np.ascontiguousarray(x[b].T).astype(BF) for b in range(B)]

    in_maps = []
    for core in range(NCORES):
        b = core // 2
        hg = core % 2
        in_maps.append({
            "xT": xTs[b],
            "wq": np.ascontiguousarray(
                w_qkv[hg * DH:(hg + 1) * DH, :].T).astype(BF),
            "wk": np.ascontiguousarray(
                w_qkv[C + hg * DH:C + (hg + 1) * DH, :].T).astype(BF),
            "wv": np.ascontiguousarray(
                w_qkv[2 * C + hg * DH:2 * C + (hg + 1) * DH, :].T).astype(BF),
            "wp": np.ascontiguousarray(
                w_proj[:, hg * DH:(hg + 1) * DH].T).astype(BF),
            "cos2": cos2,
            "sin2": sin2,
            "p2t": p2t,
        })
    return in_maps


def kernel(x, rope_cos, rope_sin, w_qkv, w_proj, b_proj, trace=False):
    from concourse.bass_utils import run_bass_kernel_spmd

    nc = build_nc()
    in_maps = make_in_maps(x, rope_cos, rope_sin, w_qkv, w_proj)
    res = run_bass_kernel_spmd(nc, in_maps, core_ids=list(range(NCORES)),
                               trace=trace)
    b_proj = np.asarray(b_proj, dtype=np.float32)
    final = np.empty((B, N, C), dtype=np.float32)
    for b in range(B):
        final[b] = res.results[2 * b]["out"] + res.results[2 * b + 1]["out"] \
            + b_proj
    if trace:
        kernel.last_exec_time_ns = res.exec_time_ns
        kernel.last_results = res
    return final


# revision 27
# speedup vs baseline: 1.2918x; 1.0029x over previous
"""Fused multi-head attention (qkv + RoPE + softmax + proj) for TRN2, 8 cores.

Sharding: core c -> batch b=c//2, head group hg=c%2 (8 of 16 heads).
Data-parallel over B (4), 2-way tensor-parallel over heads.
Host unshard: out[b] = partial[2b] + partial[2b+1] + b_proj.

v4: all matmul operands bf16 (1 cyc/col @2.4GHz).  The attention loop is
gated by the ScalarE softmax exp at ~(1024+340)/1.2GHz ~= 1.12us per key
tile; everything else is paced to hide under it:
  head:  k^T (+rope; the rot matmul is delayed one tile so the PE never
         waits on the psum->sbuf copy), v, q^T chunk 0.
  loop:  scores pair (tile_position quadrants) -> exp -> attn@v pair, plus
         a filler queue popped once per ki at 1-matmul granularity:
         remaining q^T tiles, proj tiles, and the block-boundary softmax
         normalization (reciprocal on DVE, partition-broadcast via a
         [1,64] ones matmul, psum x psum multiply into aoT on DVE).
PSUM budget: scores 2x2 banks, attn@v accumulators 2 (single-buffered),
filler accum 1, rot/broadcast 1.
"""

import sys

if "/opt/trn_rl_repo" not in sys.path:
    sys.path.insert(0, "/opt/trn_rl_repo")

import numpy as np
from contextlib import ExitStack

B, N, C, H, D = 4, 2048, 1024, 16, 64
NCORES = 8
P = 128
DH = 512          # per-core head channels (8 heads x 64)
CT = C // P       # 8 contraction tiles for qkv
DHT = DH // P     # 4 partition tiles of qT/kT/aoT
NT = N // P       # 16 n tiles
NCH = N // 512    # 4 n chunks of 512
KT = N // P       # 16 key tiles

_CACHE = {}


def _emit(nc, tc, mybir, bass, tile):
    F32 = mybir.dt.float32
    BF16 = mybir.dt.bfloat16
    FP16 = mybir.dt.float16
    Exp = mybir.ActivationFunctionType.Exp

    xT = nc.dram_tensor("xT", [C, N], BF16, kind="ExternalInput").ap()
    wq = nc.dram_tensor("wq", [C, DH], BF16, kind="ExternalInput").ap()
    wk = nc.dram_tensor("wk", [C, DH], BF16, kind="ExternalInput").ap()
    wv = nc.dram_tensor("wv", [C, DH], BF16, kind="ExternalInput").ap()
    wp = nc.dram_tensor("wp", [DH, C], BF16, kind="ExternalInput").ap()
    cos2 = nc.dram_tensor("cos2", [P, N], BF16, kind="ExternalInput").ap()
    sin2 = nc.dram_tensor("sin2", [P, N], BF16, kind="ExternalInput").ap()
    p2t = nc.dram_tensor("p2t", [P, P], BF16, kind="ExternalInput").ap()
    out = nc.dram_tensor("out", [N, C], F32, kind="ExternalOutput").ap()

    def merged(src, rows, blocks, width, off=0):
        # [blocks*128, width] dram slab -> [128, blocks, width] sbuf tile
        return bass.AP(tensor=src.tensor, offset=off,
                       ap=[[rows, P], [P * rows, blocks], [1, width]])

    ctx = ExitStack()
    with ctx:
        consts = ctx.enter_context(tc.tile_pool(name="consts", bufs=1))
        persist = ctx.enter_context(tc.tile_pool(name="persist", bufs=1))

        cos_sb = consts.tile([P, N], BF16, tag="cos")
        sin_sb = consts.tile([P, N], BF16, tag="sin")
        p2t_sb = consts.tile([P, P], BF16, tag="p2t")
        qT = [persist.tile([P, N], BF16, tag=f"qT{t}", name=f"qT{t}")
              for t in range(DHT)]
        kTt = [persist.tile([P, N], BF16, tag=f"kT{t}", name=f"kT{t}")
               for t in range(DHT)]
        v_sb = [persist.tile([P, 8 * 65], BF16, tag=f"v{i}", name=f"v{i}")
                for i in range(NT)]
        # x and wq stay resident: attention-phase filler computes qT c1..c3
        x_sb = [persist.tile([P, CT, 512], BF16, tag=f"x{i}", name=f"x{i}")
                for i in range(NCH)]
        wq_sb = persist.tile([P, CT, 512], BF16, tag="wq", name="wq")
        wp_sb = persist.tile([P, DHT, C], BF16, tag="wp", name="wp")
        aoT = [persist.tile([P, N], BF16, tag=f"aoT{t}", name=f"aoT{t}")
               for t in range(DHT)]

        p1 = ExitStack()
        wpool = p1.enter_context(tc.tile_pool(name="wkv", bufs=1))
        tpool = p1.enter_context(tc.tile_pool(name="p1tmp", bufs=3))
        qk_ps_pool = p1.enter_context(
            tc.tile_pool(name="p1ps", bufs=3, space="PSUM"))
        rot_ps_pool = p1.enter_context(
            tc.tile_pool(name="p1ps2", bufs=2, space="PSUM"))
        v_ps_pool = p1.enter_context(
            tc.tile_pool(name="p1ps3", bufs=2, space="PSUM"))

        wk_sb = wpool.tile([P, CT, 512], BF16, tag="wk", name="wk")
        wv_sb = wpool.tile([P, CT, 512], BF16, tag="wv", name="wv")

        # ---- DMA issue order: first compute needs x0+wk ----
        def load_x(nch):
            for g in range(4):
                sub = bass.AP(tensor=xT.tensor,
                              offset=(g * 2 * P) * N + nch * 512,
                              ap=[[N, P], [P * N, 2], [1, 512]])
                nc.sync.dma_start(x_sb[nch][:, 2 * g:2 * g + 2], sub)
        def load_w(dst, src, pairs=4, width=DH):
            # kc-pair sub-DMAs: short issue, parallel engines, and the
            # first accumulation matmuls only need the first pair
            for g in range(pairs):
                sub = bass.AP(tensor=src.tensor, offset=(g * 2 * P) * width,
                              ap=[[width, P], [P * width, 2], [1, width]])
                nc.sync.dma_start(dst[:, 2 * g:2 * g + 2], sub)
        load_w(wk_sb, wk)
        load_x(0)
        nc.sync.dma_start(p2t_sb, p2t)
        load_w(wq_sb, wq)
        load_x(1)
        nc.sync.dma_start(cos_sb, cos2)
        nc.sync.dma_start(sin_sb, sin2)
        load_x(2)
        load_w(wv_sb, wv)
        load_x(3)
        load_w(wp_sb, wp, pairs=2, width=C)
        for i in range(NT):   # softmax-denominator ones column of v
            ones_cols = bass.AP(
                tensor=v_sb[i].tensor, offset=64,
                ap=[list(v_sb[i].ap[0]), [65, 8]])
            nc.vector.memset(ones_cols, 1.0)

        # ---- phase 1 head: k (+rope, rot delayed 1 tile), v, q chunk 0 ----
        rot_pend = []

        def emit_rot():
            raw, dst, nsl = rot_pend.pop(0)
            rot = rot_ps_pool.tile([P, 512], F32, tag="rot_ps")
            nc.tensor.matmul(rot, p2t_sb, raw, start=True, stop=True)
            t1 = tpool.tile([P, 512], F32, tag="t1")
            nc.vector.tensor_mul(t1, raw, cos_sb[:, nsl])
            t2 = tpool.tile([P, 512], F32, tag="t2")
            nc.vector.tensor_mul(t2, rot, sin_sb[:, nsl])
            nc.vector.tensor_add(dst[:, nsl], t1, t2)

        def emit_qk_tile(w_sb, dst, t, nch):
            nsl = slice(nch * 512, (nch + 1) * 512)
            ps = qk_ps_pool.tile([P, 512], F32, tag="qk_ps")
            for kc in range(CT):
                nc.tensor.matmul(
                    ps, w_sb[:, kc, t * P:(t + 1) * P], x_sb[nch][:, kc],
                    start=(kc == 0), stop=(kc == CT - 1))
            raw = tpool.tile([P, 512], BF16, tag="raw")
            nc.scalar.copy(raw, ps)
            rot_pend.append((raw, dst, nsl))
            if len(rot_pend) > 1:
                emit_rot()

        for nch in range(NCH):
            for t in range(DHT):
                emit_qk_tile(wk_sb, kTt[t], t, nch)
        for nch in range(NCH):
            for nt4 in range(4):
                i = nch * 4 + nt4
                ps = v_ps_pool.tile([P, 512], F32, tag="v_ps")
                for kc in range(CT):
                    nc.tensor.matmul(
                        ps, x_sb[nch][:, kc, nt4 * P:(nt4 + 1) * P],
                        wv_sb[:, kc],
                        start=(kc == 0), stop=(kc == CT - 1))
                v_view = bass.AP(
                    tensor=v_sb[i].tensor, offset=0,
                    ap=[list(v_sb[i].ap[0]), [65, 8], [1, 64]])
                nc.scalar.copy(v_view, ps.rearrange("p (h d) -> p h d", h=8))
        for nch in range(NCH):
            for t in range(DHT):
                emit_qk_tile(wq_sb, qT[t], t, nch)
        while rot_pend:
            emit_rot()
        p1.close()

        # ---------------- attention + proj ----------------
        attn_ctx = ExitStack()
        epool = attn_ctx.enter_context(tc.tile_pool(name="epool2", bufs=3))
        atmp = attn_ctx.enter_context(tc.tile_pool(name="atmp", bufs=3))
        s_ps_pool = attn_ctx.enter_context(
            tc.tile_pool(name="s_ps", bufs=1, space="PSUM"))
        o_ps_pool = attn_ctx.enter_context(
            tc.tile_pool(name="o_ps", bufs=2, space="PSUM"))
        pending = []   # filler pieces (closures)

        def make_proj_pieces(nt, fc):
            # one output tile's proj as two 2-matmul PE filler pieces; the
            # psum tile is created by piece A and finished by piece B
            box = {}

            def pa():
                ps = o_ps_pool.tile([P, 512], F32, tag=f"o{nt % 2}",
                                    name="pps")
                box["ps"] = ps
                for ct in range(2):
                    nc.tensor.matmul(
                        ps, aoT[ct][:, nt * P:(nt + 1) * P],
                        wp_sb[:, ct, fc * 512:(fc + 1) * 512],
                        start=(ct == 0), stop=False)

            def pb():
                ps = box["ps"]
                for ct in range(2, DHT):
                    nc.tensor.matmul(
                        ps, aoT[ct][:, nt * P:(nt + 1) * P],
                        wp_sb[:, ct, fc * 512:(fc + 1) * 512],
                        start=False, stop=(ct == DHT - 1))
                ob = atmp.tile([P, 512], F32, tag="ob")
                nc.vector.tensor_copy(ob, ps)
                nc.sync.dma_start(
                    out[nt * P:(nt + 1) * P, fc * 512:(fc + 1) * 512], ob)
            return [pa, pb]

        def emit_norm(o_ps_par, hp, qsl, pb_):
            dd = atmp.tile([1, 512], F32, tag="dd")
            nc.vector.tensor_copy(dd, o_ps_par[64:65, :])
            r32 = atmp.tile([1, 512], F32, tag="r32")
            nc.vector.reciprocal_approx_fast(r32, dd)
            rb = atmp.tile([P, 512], F32, tag="rb")
            nc.gpsimd.partition_broadcast(rb[0:64, :], r32, channels=64)
            nc.vector.tensor_mul(
                aoT[hp][pb_:pb_ + 64, qsl], o_ps_par[0:64, :],
                rb[0:64, :])

        for qc in range(NCH):
            qsl = slice(qc * 512, (qc + 1) * 512)
            for hp in range(4):      # head pairs (even@part0-63, odd@64-127)
                o_ps = {}
                for par in range(2):  # par=0: even head, par=1: odd head
                    o_ps[par] = o_ps_pool.tile([P, 512], F32,
                                               tag=f"o{par}", name=f"o{par}")

                def emit_av(ki, e):
                    for par in range(2):
                        h = hp * 2 + par
                        # rows 0-63: attn@v; row 64: softmax denominator
                        nc.tensor.matmul(
                            o_ps[par][0:65, :],
                            v_sb[ki][:, h * 65:(h + 1) * 65],
                            e[:, par],
                            start=(ki == 0), stop=(ki == KT - 1))

                prev = None
                for ki in range(KT):
                    ksl = slice(ki * P, (ki + 1) * P)
                    s_ps = s_ps_pool.tile([P, 2, 512], F32,
                                          tag=f"s{ki % 2}", name=f"s{ki % 2}")
                    for par in range(2):
                        pb_ = par * 64
                        nc.tensor.matmul(
                            s_ps[:, par],
                            kTt[hp][pb_:pb_ + 64, ksl],
                            qT[hp][pb_:pb_ + 64, qsl],
                            start=True, stop=True,
                            tile_position=(pb_, 0))
                    if prev is not None:
                        emit_av(*prev)
                    if pending and ((ki >= 2 and ki % 3 == 2)
                                    or (qc == NCH - 1 and ki % 2 == 1)):
                        pending.pop(0)()
                    e = epool.tile([P, 2, 512], BF16, tag="e", name="e")
                    nc.scalar.activation(e, s_ps, Exp,
                                         scale=float(D) ** -0.5)
                    prev = (ki, e)
                emit_av(*prev)
                for par in range(2):
                    emit_norm(o_ps[par], hp, qsl, par * 64)
            for nt in range(qc * 4, (qc + 1) * 4):
                for fc in range(2):
                    pending.extend(make_proj_pieces(nt, fc))
        for fn in pending:
            fn()
        attn_ctx.close()


def build_nc():
    if "nc" in _CACHE:
        return _CACHE["nc"]
    import concourse.bass as bass
    import concourse.tile as tile
    from concourse import bacc, mybir

    nc = bacc.Bacc("TRN2", target_bir_lowering=False, debug=False,
                   enable_asserts=False, num_devices=NCORES)
    with tile.TileContext(nc) as tc:
        _emit(nc, tc, mybir, bass, tile)
    nc.compile()
    _CACHE["nc"] = nc
    return nc


def make_in_maps(x, rope_cos, rope_sin, w_qkv, w_proj):
    import ml_dtypes
    BF = ml_dtypes.bfloat16

    x = np.asarray(x, dtype=np.float32)
    rope_cos = np.asarray(rope_cos, dtype=np.float32)
    rope_sin = np.asarray(rope_sin, dtype=np.float32)
    w_qkv = np.asarray(w_qkv, dtype=np.float32)
    w_proj = np.asarray(w_proj, dtype=np.float32)

    cosT = np.ascontiguousarray(rope_cos.T)           # [64, N]
    cos2 = np.vstack([cosT, cosT]).astype(BF)         # [128, N]
    sinT = np.ascontiguousarray(rope_sin.T)
    sin2 = np.vstack([sinT, sinT]).astype(BF)

    # signed half-rotation permutation: rot(q) = P2 @ q (per 64-block)
    R = np.zeros((D, D), dtype=np.float32)
    half = D // 2
    R[np.arange(half), np.arange(half) + half] = -1.0
    R[np.arange(half) + half, np.arange(half)] = 1.0
    P2 = np.zeros((P, P), dtype=np.float32)
    P2[:D, :D] = R
    P2[D:, D:] = R
    p2t = np.ascontiguousarray(P2.T).astype(BF)

    xTs = [np.ascontiguousarray(x[b].T).astype(BF) for b in range(B)]

    in_maps = []
    for core in range(NCORES):
        b = core // 2
        hg = core % 2
        in_maps.append({
            "xT": xTs[b],
            "wq": np.ascontiguousarray(
                w_qkv[hg * DH:(hg + 1) * DH, :].T).astype(BF),
            "wk": np.ascontiguousarray(
                w_qkv[C + hg * DH:C + (hg + 1) * DH, :].T).astype(BF),
            "wv": np.ascontiguousarray(
                w_qkv[2 * C + hg * DH:2 * C + (hg + 1) * DH, :].T).astype(BF),
            "wp": np.ascontiguousarray(
                w_proj[:, hg * DH:(hg + 1) * DH].T).astype(BF),
            "cos2": cos2,
            "sin2": sin2,
            "p2t": p2t,
        })
    return in_maps


def kernel(x, rope_cos, rope_sin, w_qkv, w_proj, b_proj, trace=False):
    from concourse.bass_utils import run_bass_kernel_spmd

    nc = build_nc()
    in_maps = make_in_maps(x, rope_cos, rope_sin, w_qkv, w_proj)
    res = run_bass_kernel_spmd(nc, in_maps, core_ids=list(range(NCORES)),
                               trace=trace)
    b_proj = np.asarray(b_proj, dtype=np.float32)
    final = np.empty((B, N, C), dtype=np.float32)
    for b in range(B):
        final[b] = res.results[2 * b]["out"] + res.results[2 * b + 1]["out"] \
            + b_proj
    if trace:
        kernel.last_exec_time_ns = res.exec_time_ns
        kernel.last_results = res
    return final


# revision 29
# speedup vs baseline: 1.2944x; 1.0020x over previous
"""Fused multi-head attention (qkv + RoPE + softmax + proj) for TRN2, 8 cores.

Sharding: core c -> batch b=c//2, head group hg=c%2 (8 of 16 heads).
Data-parallel over B (4), 2-way tensor-parallel over heads.
Host unshard: out[b] = partial[2b] + partial[2b+1] + b_proj.

v4: all matmul operands bf16 (1 cyc/col @2.4GHz).  The attention loop is
gated by the ScalarE softmax exp at ~(1024+340)/1.2GHz ~= 1.12us per key
tile; everything else is paced to hide under it:
  head:  k^T (+rope; the rot matmul is delayed one tile so the PE never
         waits on the psum->sbuf copy), v, q^T chunk 0.
  loop:  scores pair (tile_position quadrants) -> exp -> attn@v pair, plus
         a filler queue popped once per ki at 1-matmul granularity:
         remaining q^T tiles, proj tiles, and the block-boundary softmax
         normalization (reciprocal on DVE, partition-broadcast via a
         [1,64] ones matmul, psum x psum multiply into aoT on DVE).
PSUM budget: scores 2x2 banks, attn@v accumulators 2 (single-buffered),
filler accum 1, rot/broadcast 1.
"""

import sys

if "/opt/trn_rl_repo" not in sys.path:
    sys.path.insert(0, "/opt/trn_rl_repo")

import numpy as np
from contextlib import ExitStack

B, N, C, H, D = 4, 2048, 1024, 16, 64
NCORES = 8
P = 128
DH = 512          # per-core head channels (8 heads x 64)
CT = C // P       # 8 contraction tiles for qkv
DHT = DH // P     # 4 partition tiles of qT/kT/aoT
NT = N // P       # 16 n tiles
NCH = N // 512    # 4 n chunks of 512
KT = N // P       # 16 key tiles

_CACHE = {}


def _emit(nc, tc, mybir, bass, tile):
    F32 = mybir.dt.float32
    BF16 = mybir.dt.bfloat16
    FP16 = mybir.dt.float16
    Exp = mybir.ActivationFunctionType.Exp

    xT = nc.dram_tensor("xT", [C, N], BF16, kind="ExternalInput").ap()
    wq = nc.dram_tensor("wq", [C, DH], BF16, kind="ExternalInput").ap()
    wk = nc.dram_tensor("wk", [C, DH], BF16, kind="ExternalInput").ap()
    wv = nc.dram_tensor("wv", [C, DH], BF16, kind="ExternalInput").ap()
    wp = nc.dram_tensor("wp", [DH, C], BF16, kind="ExternalInput").ap()
    cos2 = nc.dram_tensor("cos2", [P, N], BF16, kind="ExternalInput").ap()
    sin2 = nc.dram_tensor("sin2", [P, N], BF16, kind="ExternalInput").ap()
    p2t = nc.dram_tensor("p2t", [P, P], BF16, kind="ExternalInput").ap()
    out = nc.dram_tensor("out", [N, C], F32, kind="ExternalOutput").ap()

    def merged(src, rows, blocks, width, off=0):
        # [blocks*128, width] dram slab -> [128, blocks, width] sbuf tile
        return bass.AP(tensor=src.tensor, offset=off,
                       ap=[[rows, P], [P * rows, blocks], [1, width]])

    ctx = ExitStack()
    with ctx:
        consts = ctx.enter_context(tc.tile_pool(name="consts", bufs=1))
        persist = ctx.enter_context(tc.tile_pool(name="persist", bufs=1))

        cos_sb = consts.tile([P, N], BF16, tag="cos")
        sin_sb = consts.tile([P, N], BF16, tag="sin")
        p2t_sb = consts.tile([P, P], BF16, tag="p2t")
        qT = [persist.tile([P, N], BF16, tag=f"qT{t}", name=f"qT{t}")
              for t in range(DHT)]
        kTt = [persist.tile([P, N], BF16, tag=f"kT{t}", name=f"kT{t}")
               for t in range(DHT)]
        v_sb = [persist.tile([P, 8 * 65], BF16, tag=f"v{i}", name=f"v{i}")
                for i in range(NT)]
        # x and wq stay resident: attention-phase filler computes qT c1..c3
        x_sb = [persist.tile([P, CT, 512], BF16, tag=f"x{i}", name=f"x{i}")
                for i in range(NCH)]
        wq_sb = persist.tile([P, CT, 512], BF16, tag="wq", name="wq")
        wp_sb = persist.tile([P, DHT, C], BF16, tag="wp", name="wp")
        aoT = [persist.tile([P, N], BF16, tag=f"aoT{t}", name=f"aoT{t}")
               for t in range(DHT)]

        p1 = ExitStack()
        wpool = p1.enter_context(tc.tile_pool(name="wkv", bufs=1))
        tpool = p1.enter_context(tc.tile_pool(name="p1tmp", bufs=3))
        qk_ps_pool = p1.enter_context(
            tc.tile_pool(name="p1ps", bufs=3, space="PSUM"))
        rot_ps_pool = p1.enter_context(
            tc.tile_pool(name="p1ps2", bufs=2, space="PSUM"))
        v_ps_pool = p1.enter_context(
            tc.tile_pool(name="p1ps3", bufs=2, space="PSUM"))

        wk_sb = wpool.tile([P, CT, 512], BF16, tag="wk", name="wk")
        wv_sb = wpool.tile([P, CT, 512], BF16, tag="wv", name="wv")

        # ---- DMA issue order: first compute needs x0+wk ----
        def load_x(nch):
            for g in range(4):
                sub = bass.AP(tensor=xT.tensor,
                              offset=(g * 2 * P) * N + nch * 512,
                              ap=[[N, P], [P * N, 2], [1, 512]])
                nc.sync.dma_start(x_sb[nch][:, 2 * g:2 * g + 2], sub)
        def load_w(dst, src, pairs=4, width=DH):
            # kc-pair sub-DMAs: short issue, parallel engines, and the
            # first accumulation matmuls only need the first pair
            for g in range(pairs):
                sub = bass.AP(tensor=src.tensor, offset=(g * 2 * P) * width,
                              ap=[[width, P], [P * width, 2], [1, width]])
                nc.sync.dma_start(dst[:, 2 * g:2 * g + 2], sub)
        def load_w_g(dst, src, g, width=DH):
            sub = bass.AP(tensor=src.tensor, offset=(g * 2 * P) * width,
                          ap=[[width, P], [P * width, 2], [1, width]])
            nc.sync.dma_start(dst[:, 2 * g:2 * g + 2], sub)

        def load_x_g(nch, g):
            sub = bass.AP(tensor=xT.tensor,
                          offset=(g * 2 * P) * N + nch * 512,
                          ap=[[N, P], [P * N, 2], [1, 512]])
            nc.sync.dma_start(x_sb[nch][:, 2 * g:2 * g + 2], sub)

        for g in range(4):   # interleave: first matmuls need wk+x0 pair 0
            load_w_g(wk_sb, wk, g)
            load_x_g(0, g)
        nc.sync.dma_start(p2t_sb, p2t)
        load_w(wq_sb, wq)
        load_x(1)
        nc.sync.dma_start(cos_sb, cos2)
        nc.sync.dma_start(sin_sb, sin2)
        load_x(2)
        load_w(wv_sb, wv)
        load_x(3)
        load_w(wp_sb, wp, pairs=2, width=C)
        for i in range(NT):   # softmax-denominator ones column of v
            ones_cols = bass.AP(
                tensor=v_sb[i].tensor, offset=64,
                ap=[list(v_sb[i].ap[0]), [65, 8]])
            nc.vector.memset(ones_cols, 1.0)

        # ---- phase 1 head: k (+rope, rot delayed 1 tile), v, q chunk 0 ----
        rot_pend = []

        def emit_rot():
            raw, dst, nsl = rot_pend.pop(0)
            rot = rot_ps_pool.tile([P, 512], F32, tag="rot_ps")
            nc.tensor.matmul(rot, p2t_sb, raw, start=True, stop=True)
            t1 = tpool.tile([P, 512], F32, tag="t1")
            nc.vector.tensor_mul(t1, raw, cos_sb[:, nsl])
            t2 = tpool.tile([P, 512], F32, tag="t2")
            nc.vector.tensor_mul(t2, rot, sin_sb[:, nsl])
            nc.vector.tensor_add(dst[:, nsl], t1, t2)

        def emit_qk_tile(w_sb, dst, t, nch):
            nsl = slice(nch * 512, (nch + 1) * 512)
            ps = qk_ps_pool.tile([P, 512], F32, tag="qk_ps")
            for kc in range(CT):
                nc.tensor.matmul(
                    ps, w_sb[:, kc, t * P:(t + 1) * P], x_sb[nch][:, kc],
                    start=(kc == 0), stop=(kc == CT - 1))
            raw = tpool.tile([P, 512], BF16, tag="raw")
            nc.scalar.copy(raw, ps)
            rot_pend.append((raw, dst, nsl))
            if len(rot_pend) > 1:
                emit_rot()

        for nch in range(NCH):
            for t in range(DHT):
                emit_qk_tile(wk_sb, kTt[t], t, nch)
        for nch in range(NCH):
            for nt4 in range(4):
                i = nch * 4 + nt4
                ps = v_ps_pool.tile([P, 512], F32, tag="v_ps")
                for kc in range(CT):
                    nc.tensor.matmul(
                        ps, x_sb[nch][:, kc, nt4 * P:(nt4 + 1) * P],
                        wv_sb[:, kc],
                        start=(kc == 0), stop=(kc == CT - 1))
                v_view = bass.AP(
                    tensor=v_sb[i].tensor, offset=0,
                    ap=[list(v_sb[i].ap[0]), [65, 8], [1, 64]])
                nc.scalar.copy(v_view, ps.rearrange("p (h d) -> p h d", h=8))
        for nch in range(NCH):
            for t in range(DHT):
                emit_qk_tile(wq_sb, qT[t], t, nch)
        while rot_pend:
            emit_rot()
        p1.close()

        # ---------------- attention + proj ----------------
        attn_ctx = ExitStack()
        epool = attn_ctx.enter_context(tc.tile_pool(name="epool2", bufs=3))
        atmp = attn_ctx.enter_context(tc.tile_pool(name="atmp", bufs=3))
        s_ps_pool = attn_ctx.enter_context(
            tc.tile_pool(name="s_ps", bufs=1, space="PSUM"))
        o_ps_pool = attn_ctx.enter_context(
            tc.tile_pool(name="o_ps", bufs=2, space="PSUM"))
        pending = []   # filler pieces (closures)

        def make_proj_pieces(nt, fc):
            # one output tile's proj as two 2-matmul PE filler pieces; the
            # psum tile is created by piece A and finished by piece B
            box = {}

            def pa():
                ps = o_ps_pool.tile([P, 512], F32, tag=f"o{nt % 2}",
                                    name="pps")
                box["ps"] = ps
                for ct in range(2):
                    nc.tensor.matmul(
                        ps, aoT[ct][:, nt * P:(nt + 1) * P],
                        wp_sb[:, ct, fc * 512:(fc + 1) * 512],
                        start=(ct == 0), stop=False)

            def pb():
                ps = box["ps"]
                for ct in range(2, DHT):
                    nc.tensor.matmul(
                        ps, aoT[ct][:, nt * P:(nt + 1) * P],
                        wp_sb[:, ct, fc * 512:(fc + 1) * 512],
                        start=False, stop=(ct == DHT - 1))
                ob = atmp.tile([P, 512], F32, tag="ob")
                nc.vector.tensor_copy(ob, ps)
                nc.sync.dma_start(
                    out[nt * P:(nt + 1) * P, fc * 512:(fc + 1) * 512], ob)
            return [pa, pb]

        def emit_norm(o_ps_par, hp, qsl, pb_):
            dd = atmp.tile([1, 512], F32, tag="dd")
            nc.vector.tensor_copy(dd, o_ps_par[64:65, :])
            r32 = atmp.tile([1, 512], F32, tag="r32")
            nc.vector.reciprocal_approx_fast(r32, dd)
            rb = atmp.tile([P, 512], F32, tag="rb")
            nc.gpsimd.partition_broadcast(rb[0:64, :], r32, channels=64)
            nc.vector.tensor_mul(
                aoT[hp][pb_:pb_ + 64, qsl], o_ps_par[0:64, :],
                rb[0:64, :])

        for qc in range(NCH):
            qsl = slice(qc * 512, (qc + 1) * 512)
            for hp in range(4):      # head pairs (even@part0-63, odd@64-127)
                o_ps = {}
                for par in range(2):  # par=0: even head, par=1: odd head
                    o_ps[par] = o_ps_pool.tile([P, 512], F32,
                                               tag=f"o{par}", name=f"o{par}")

                def emit_av(ki, e):
                    for par in range(2):
                        h = hp * 2 + par
                        # rows 0-63: attn@v; row 64: softmax denominator
                        nc.tensor.matmul(
                            o_ps[par][0:65, :],
                            v_sb[ki][:, h * 65:(h + 1) * 65],
                            e[:, par],
                            start=(ki == 0), stop=(ki == KT - 1))

                prev = None
                for ki in range(KT):
                    ksl = slice(ki * P, (ki + 1) * P)
                    s_ps = s_ps_pool.tile([P, 2, 512], F32,
                                          tag=f"s{ki % 2}", name=f"s{ki % 2}")
                    for par in range(2):
                        pb_ = par * 64
                        nc.tensor.matmul(
                            s_ps[:, par],
                            kTt[hp][pb_:pb_ + 64, ksl],
                            qT[hp][pb_:pb_ + 64, qsl],
                            start=True, stop=True,
                            tile_position=(pb_, 0))
                    if prev is not None:
                        emit_av(*prev)
                    # keep a small reserve so dependency-free pieces remain
                    # to bridge the end-of-loop normalize chain (HAM-warm)
                    if len(pending) > 4 and ((ki >= 2 and ki % 3 == 2)
                                             or (qc == NCH - 1
                                                 and ki % 2 == 1)):
                        pending.pop(0)()
                    e = epool.tile([P, 2, 512], BF16, tag="e", name="e")
                    nc.scalar.activation(e, s_ps, Exp,
                                         scale=float(D) ** -0.5)
                    prev = (ki, e)
                emit_av(*prev)
                for par in range(2):
                    emit_norm(o_ps[par], hp, qsl, par * 64)
            for nt in range(qc * 4, (qc + 1) * 4):
                for fc in range(2):
                    pending.extend(make_proj_pieces(nt, fc))
        for fn in pending:
            fn()
        attn_ctx.close()


def build_nc():
    if "nc" in _CACHE:
        return _CACHE["nc"]
    import concourse.bass as bass
    import concourse.tile as tile
    from concourse import bacc, mybir

    nc = bacc.Bacc("TRN2", target_bir_lowering=False, debug=False,
                   enable_asserts=False, num_devices=NCORES)
    with tile.TileContext(nc) as tc:
        _emit(nc, tc, mybir, bass, tile)
    nc.compile()
    _CACHE["nc"] = nc
    return nc


def make_in_maps(x, rope_cos, rope_sin, w_qkv, w_proj):
    import ml_dtypes
    BF = ml_dtypes.bfloat16

    x = np.asarray(x, dtype=np.float32)
    rope_cos = np.asarray(rope_cos, dtype=np.float32)
    rope_sin = np.asarray(rope_sin, dtype=np.float32)
    w_qkv = np.asarray(w_qkv, dtype=np.float32)
    w_proj = np.asarray(w_proj, dtype=np.float32)

    cosT = np.ascontiguousarray(rope_cos.T)           # [64, N]
    cos2 = np.vstack([cosT, cosT]).astype(BF)         # [128, N]
    sinT = np.ascontiguousarray(rope_sin.T)
    sin2 = np.vstack([sinT, sinT]).astype(BF)

    # signed half-rotation permutation: rot(q) = P2 @ q (per 64-block)
    R = np.zeros((D, D), dtype=np.float32)
    half = D // 2
    R[np.arange(half), np.arange(half) + half] = -1.0
    R[np.arange(half) + half, np.arange(half)] = 1.0
    P2 = np.zeros((P, P), dtype=np.float32)
    P2[:D, :D] = R
    P2[D:, D:] = R
    p2t = np.ascontiguousarray(P2.T).astype(BF)

    xTs = [np.ascontiguousarray(x[b].T).astype(BF) for b in range(B)]

    in_maps = []
    for core in range(NCORES):
        b = core // 2
        hg = core % 2
        in_maps.append({
            "xT": xTs[b],
            "wq": np.ascontiguousarray(
                w_qkv[hg * DH:(hg + 1) * DH, :].T).astype(BF),
            "wk": np.ascontiguousarray(
                w_qkv[C + hg * DH:C + (hg + 1) * DH, :].T).astype(BF),
            "wv": np.ascontiguousarray(
                w_qkv[2 * C + hg * DH:2 * C + (hg + 1) * DH, :].T).astype(BF),
            "wp": np.ascontiguousarray(
                w_proj[:, hg * DH:(hg + 1) * DH].T).astype(BF),
            "cos2": cos2,
            "sin2": sin2,
            "p2t": p2t,
        })
    return in_maps


def kernel(x, rope_cos, rope_sin, w_qkv, w_proj, b_proj, trace=False):
    from concourse.bass_utils import run_bass_kernel_spmd

    nc = build_nc()
    in_maps = make_in_maps(x, rope_cos, rope_sin, w_qkv, w_proj)
    res = run_bass_kernel_spmd(nc, in_maps, core_ids=list(range(NCORES)),
                               trace=trace)
    b_proj = np.asarray(b_proj, dtype=np.float32)
    final = np.empty((B, N, C), dtype=np.float32)
    for b in range(B):
        final[b] = res.results[2 * b]["out"] + res.results[2 * b + 1]["out"] \
            + b_proj
    if trace:
        kernel.last_exec_time_ns = res.exec_time_ns
        kernel.last_results = res
    return final


# revision 36
# speedup vs baseline: 1.3031x; 1.0067x over previous
"""Fused multi-head attention (qkv + RoPE + softmax + proj) for TRN2, 8 cores.

Sharding: core c -> batch b=c//2, head group hg=c%2 (8 of 16 heads).
Data-parallel over B (4), 2-way tensor-parallel over heads.
Host unshard: out[b] = partial[2b] + partial[2b+1] + b_proj.

v4: all matmul operands bf16 (1 cyc/col @2.4GHz).  The attention loop is
gated by the ScalarE softmax exp at ~(1024+340)/1.2GHz ~= 1.12us per key
tile; everything else is paced to hide under it:
  head:  k^T (+rope; the rot matmul is delayed one tile so the PE never
         waits on the psum->sbuf copy), v, q^T chunk 0.
  loop:  scores pair (tile_position quadrants) -> exp -> attn@v pair, plus
         a filler queue popped once per ki at 1-matmul granularity:
         remaining q^T tiles, proj tiles, and the block-boundary softmax
         normalization (reciprocal on DVE, partition-broadcast via a
         [1,64] ones matmul, psum x psum multiply into aoT on DVE).
PSUM budget: scores 2x2 banks, attn@v accumulators 2 (single-buffered),
filler accum 1, rot/broadcast 1.
"""

import sys

if "/opt/trn_rl_repo" not in sys.path:
    sys.path.insert(0, "/opt/trn_rl_repo")

import numpy as np
from contextlib import ExitStack

B, N, C, H, D = 4, 2048, 1024, 16, 64
NCORES = 8
P = 128
DH = 512          # per-core head channels (8 heads x 64)
CT = C // P       # 8 contraction tiles for qkv
DHT = DH // P     # 4 partition tiles of qT/kT/aoT
NT = N // P       # 16 n tiles
NCH = N // 512    # 4 n chunks of 512
KT = N // P       # 16 key tiles

_CACHE = {}


def _emit(nc, tc, mybir, bass, tile):
    F32 = mybir.dt.float32
    BF16 = mybir.dt.bfloat16
    FP16 = mybir.dt.float16
    Exp = mybir.ActivationFunctionType.Exp

    xT = nc.dram_tensor("xT", [C, N], BF16, kind="ExternalInput").ap()
    wq = nc.dram_tensor("wq", [C, DH], BF16, kind="ExternalInput").ap()
    wk = nc.dram_tensor("wk", [C, DH], BF16, kind="ExternalInput").ap()
    wv = nc.dram_tensor("wv", [C, DH], BF16, kind="ExternalInput").ap()
    wp = nc.dram_tensor("wp", [DH, C], BF16, kind="ExternalInput").ap()
    cos2 = nc.dram_tensor("cos2", [P, N], BF16, kind="ExternalInput").ap()
    sin2 = nc.dram_tensor("sin2", [P, N], BF16, kind="ExternalInput").ap()
    p2t = nc.dram_tensor("p2t", [P, P], BF16, kind="ExternalInput").ap()
    out = nc.dram_tensor("out", [N, C], F32, kind="ExternalOutput").ap()

    def merged(src, rows, blocks, width, off=0):
        # [blocks*128, width] dram slab -> [128, blocks, width] sbuf tile
        return bass.AP(tensor=src.tensor, offset=off,
                       ap=[[rows, P], [P * rows, blocks], [1, width]])

    ctx = ExitStack()
    with ctx:
        consts = ctx.enter_context(tc.tile_pool(name="consts", bufs=1))
        persist = ctx.enter_context(tc.tile_pool(name="persist", bufs=1))

        cos_sb = consts.tile([P, N], BF16, tag="cos")
        sin_sb = consts.tile([P, N], BF16, tag="sin")
        p2t_sb = consts.tile([P, P], BF16, tag="p2t")
        qT = [persist.tile([P, N], BF16, tag=f"qT{t}", name=f"qT{t}")
              for t in range(DHT)]
        kTt = [persist.tile([P, N], BF16, tag=f"kT{t}", name=f"kT{t}")
               for t in range(DHT)]
        v_sb = [persist.tile([P, 8 * 65], BF16, tag=f"v{i}", name=f"v{i}")
                for i in range(NT)]
        # x and wq stay resident: attention-phase filler computes qT c1..c3
        x_sb = [persist.tile([P, CT, 512], BF16, tag=f"x{i}", name=f"x{i}")
                for i in range(NCH)]
        wq_sb = persist.tile([P, CT, 512], BF16, tag="wq", name="wq")
        wp_sb = persist.tile([P, DHT, C], BF16, tag="wp", name="wp")
        aoT = [persist.tile([P, N], BF16, tag=f"aoT{t}", name=f"aoT{t}")
               for t in range(DHT)]

        p1 = ExitStack()
        wpool = p1.enter_context(tc.tile_pool(name="wkv", bufs=1))
        tpool = p1.enter_context(tc.tile_pool(name="p1tmp", bufs=3))
        qk_ps_pool = p1.enter_context(
            tc.tile_pool(name="p1ps", bufs=3, space="PSUM"))
        rot_ps_pool = p1.enter_context(
            tc.tile_pool(name="p1ps2", bufs=2, space="PSUM"))
        v_ps_pool = p1.enter_context(
            tc.tile_pool(name="p1ps3", bufs=2, space="PSUM"))

        wk_sb = wpool.tile([P, CT, 512], BF16, tag="wk", name="wk")
        wv_sb = wpool.tile([P, CT, 512], BF16, tag="wv", name="wv")

        # ---- DMA issue order: first compute needs x0+wk ----
        def load_x(nch):
            for g in range(4):
                sub = bass.AP(tensor=xT.tensor,
                              offset=(g * 2 * P) * N + nch * 512,
                              ap=[[N, P], [P * N, 2], [1, 512]])
                nc.sync.dma_start(x_sb[nch][:, 2 * g:2 * g + 2], sub)
        def load_w(dst, src, pairs=4, width=DH):
            # kc-pair sub-DMAs: short issue, parallel engines, and the
            # first accumulation matmuls only need the first pair
            for g in range(pairs):
                sub = bass.AP(tensor=src.tensor, offset=(g * 2 * P) * width,
                              ap=[[width, P], [P * width, 2], [1, width]])
                nc.sync.dma_start(dst[:, 2 * g:2 * g + 2], sub)
        def load_w_g(dst, src, g, width=DH):
            sub = bass.AP(tensor=src.tensor, offset=(g * 2 * P) * width,
                          ap=[[width, P], [P * width, 2], [1, width]])
            nc.sync.dma_start(dst[:, 2 * g:2 * g + 2], sub)

        def load_x_g(nch, g):
            sub = bass.AP(tensor=xT.tensor,
                          offset=(g * 2 * P) * N + nch * 512,
                          ap=[[N, P], [P * N, 2], [1, 512]])
            nc.sync.dma_start(x_sb[nch][:, 2 * g:2 * g + 2], sub)

        for g in range(4):   # interleave: first matmuls need wk+x0 pair 0
            load_w_g(wk_sb, wk, g)
            load_x_g(0, g)
        nc.sync.dma_start(p2t_sb, p2t)
        load_w(wq_sb, wq)
        load_x(1)
        nc.sync.dma_start(cos_sb, cos2)
        nc.sync.dma_start(sin_sb, sin2)
        load_x(2)
        load_w(wv_sb, wv)
        load_x(3)
        load_w(wp_sb, wp, pairs=2, width=C)
        for i in range(NT):   # softmax-denominator ones column of v
            ones_cols = bass.AP(
                tensor=v_sb[i].tensor, offset=64,
                ap=[list(v_sb[i].ap[0]), [65, 8]])
            nc.vector.memset(ones_cols, 1.0)

        # ---- phase 1 head: k (+rope, rot delayed 1 tile), v, q chunk 0 ----
        rot_pend = []

        def emit_rot():
            raw, dst, nsl = rot_pend.pop(0)
            rot = rot_ps_pool.tile([P, 512], F32, tag="rot_ps")
            nc.tensor.matmul(rot, p2t_sb, raw, start=True, stop=True)
            t1 = tpool.tile([P, 512], F32, tag="t1")
            nc.vector.tensor_mul(t1, raw, cos_sb[:, nsl])
            t2 = tpool.tile([P, 512], F32, tag="t2")
            nc.vector.tensor_mul(t2, rot, sin_sb[:, nsl])
            nc.vector.tensor_add(dst[:, nsl], t1, t2)

        def emit_qk_tile(w_sb, dst, t, nch):
            nsl = slice(nch * 512, (nch + 1) * 512)
            ps = qk_ps_pool.tile([P, 512], F32, tag="qk_ps")
            for kc in range(CT):
                nc.tensor.matmul(
                    ps, w_sb[:, kc, t * P:(t + 1) * P], x_sb[nch][:, kc],
                    start=(kc == 0), stop=(kc == CT - 1))
            raw = tpool.tile([P, 512], BF16, tag="raw")
            nc.scalar.copy(raw, ps)
            rot_pend.append((raw, dst, nsl))
            if len(rot_pend) > 1:
                emit_rot()

        for nch in range(NCH):
            for t in range(DHT):
                emit_qk_tile(wk_sb, kTt[t], t, nch)
        for nch in range(NCH):
            for nt4 in range(4):
                i = nch * 4 + nt4
                ps = v_ps_pool.tile([P, 512], F32, tag="v_ps")
                for kc in range(CT):
                    nc.tensor.matmul(
                        ps, x_sb[nch][:, kc, nt4 * P:(nt4 + 1) * P],
                        wv_sb[:, kc],
                        start=(kc == 0), stop=(kc == CT - 1))
                v_view = bass.AP(
                    tensor=v_sb[i].tensor, offset=0,
                    ap=[list(v_sb[i].ap[0]), [65, 8], [1, 64]])
                nc.scalar.copy(v_view, ps.rearrange("p (h d) -> p h d", h=8))
        for nch in range(NCH):
            for t in range(DHT):
                emit_qk_tile(wq_sb, qT[t], t, nch)
        while rot_pend:
            emit_rot()
        p1.close()

        # ---------------- attention + proj ----------------
        attn_ctx = ExitStack()
        epool = attn_ctx.enter_context(tc.tile_pool(name="epool2", bufs=3))
        atmp = attn_ctx.enter_context(tc.tile_pool(name="atmp", bufs=3))
        s_ps_pool = attn_ctx.enter_context(
            tc.tile_pool(name="s_ps", bufs=1, space="PSUM"))
        o_ps_pool = attn_ctx.enter_context(
            tc.tile_pool(name="o_ps", bufs=2, space="PSUM"))
        pending = []   # filler pieces (closures)

        def make_proj_pieces(nt, fc):
            # one output tile's proj as two 2-matmul PE filler pieces; the
            # psum tile is created by piece A and finished by piece B
            box = {}

            def pa():
                ps = o_ps_pool.tile([P, 512], F32, tag=f"o{nt % 2}",
                                    name="pps")
                box["ps"] = ps
                for ct in range(2):
                    nc.tensor.matmul(
                        ps, aoT[ct][:, nt * P:(nt + 1) * P],
                        wp_sb[:, ct, fc * 512:(fc + 1) * 512],
                        start=(ct == 0), stop=False)

            def pb():
                ps = box["ps"]
                for ct in range(2, DHT):
                    nc.tensor.matmul(
                        ps, aoT[ct][:, nt * P:(nt + 1) * P],
                        wp_sb[:, ct, fc * 512:(fc + 1) * 512],
                        start=False, stop=(ct == DHT - 1))
                ob = atmp.tile([P, 512], F32, tag="ob")
                nc.vector.tensor_copy(ob, ps)
                nc.sync.dma_start(
                    out[nt * P:(nt + 1) * P, fc * 512:(fc + 1) * 512], ob)
            return [pa, pb]

        def emit_norm(o_ps_par, hp, qsl, pb_):
            dd = atmp.tile([1, 512], F32, tag="dd")
            nc.vector.tensor_copy(dd, o_ps_par[64:65, :])
            r32 = atmp.tile([1, 512], F32, tag="r32")
            nc.vector.reciprocal_approx_fast(r32, dd)
            rb = atmp.tile([P, 512], F32, tag="rb")
            nc.gpsimd.partition_broadcast(rb[0:64, :], r32, channels=64)
            nc.vector.tensor_mul(
                aoT[hp][pb_:pb_ + 64, qsl], o_ps_par[0:64, :],
                rb[0:64, :])

        for qc in range(NCH):
            qsl = slice(qc * 512, (qc + 1) * 512)
            for hp in range(4):      # head pairs (even@part0-63, odd@64-127)
                o_ps = {}
                for par in range(2):  # par=0: even head, par=1: odd head
                    o_ps[par] = o_ps_pool.tile([P, 512], F32,
                                               tag=f"o{par}", name=f"o{par}")

                def emit_av(ki, e):
                    for par in range(2):
                        h = hp * 2 + par
                        # rows 0-63: attn@v; row 64: softmax denominator
                        nc.tensor.matmul(
                            o_ps[par][0:65, :],
                            v_sb[ki][:, h * 65:(h + 1) * 65],
                            e[:, par],
                            start=(ki == 0), stop=(ki == KT - 1))

                prev = None
                for ki in range(KT):
                    ksl = slice(ki * P, (ki + 1) * P)
                    s_ps = s_ps_pool.tile([P, 2, 512], F32,
                                          tag=f"s{ki % 2}", name=f"s{ki % 2}")
                    for par in range(2):
                        pb_ = par * 64
                        nc.tensor.matmul(
                            s_ps[:, par],
                            kTt[hp][pb_:pb_ + 64, ksl],
                            qT[hp][pb_:pb_ + 64, qsl],
                            start=True, stop=True,
                            tile_position=(pb_, 0))
                    if prev is not None:
                        emit_av(*prev)
                    # keep a small reserve so dependency-free pieces remain
                    # to bridge the end-of-loop normalize chain (HAM-warm)
                    if len(pending) > 4 and ((ki >= 2 and ki % 3 == 2)
                                             or (qc == NCH - 1
                                                 and ki % 2 == 1)):
                        pending.pop(0)()
                    e = epool.tile([P, 2, 512], BF16, tag="e", name="e")
                    nc.scalar.activation(e, s_ps, Exp,
                                         scale=float(D) ** -0.5)
                    prev = (ki, e)
                emit_av(*prev)
                for par in range(2):
                    emit_norm(o_ps[par], hp, qsl, par * 64)
            for nt in range(qc * 4, (qc + 1) * 4):
                for fc in range(2):
                    pending.extend(make_proj_pieces(nt, fc))
        for fn in pending:
            fn()
        attn_ctx.close()


def build_nc():
    if "nc" in _CACHE:
        return _CACHE["nc"]
    import concourse.bass as bass
    import concourse.tile as tile
    from concourse import bacc, mybir

    nc = bacc.Bacc("TRN2", target_bir_lowering=False, debug=False,
                   enable_asserts=False, num_devices=NCORES)
    with tile.TileContext(nc) as tc:
        _emit(nc, tc, mybir, bass, tile)
    nc.compile()
    _CACHE["nc"] = nc
    return nc


def make_in_maps(x, rope_cos, rope_sin, w_qkv, w_proj):
    import ml_dtypes
    BF = ml_dtypes.bfloat16

    x = np.asarray(x, dtype=np.float32)
    rope_cos = np.asarray(rope_cos, dtype=np.float32)
    rope_sin = np.asarray(rope_sin, dtype=np.float32)
    w_qkv = np.asarray(w_qkv, dtype=np.float32)
    w_proj = np.asarray(w_proj, dtype=np.float32)

    cosT = np.ascontiguousarray(rope_cos.T)           # [64, N]
    cos2 = np.vstack([cosT, cosT]).astype(BF)         # [128, N]
    sinT = np.ascontiguousarray(rope_sin.T)
    sin2 = np.vstack([sinT, sinT]).astype(BF)

    # signed half-rotation permutation: rot(q) = P2 @ q (per 64-block)
    R = np.zeros((D, D), dtype=np.float32)
    half = D // 2
    R[np.arange(half), np.arange(half) + half] = -1.0
    R[np.arange(half) + half, np.arange(half)] = 1.0
    P2 = np.zeros((P, P), dtype=np.float32)
    P2[:D, :D] = R
    P2[D:, D:] = R
    p2t = np.ascontiguousarray(P2.T).astype(BF)

    xTs = [np.ascontiguousarray(x[b].T).astype(BF) for b in range(B)]

    in_maps = []
    for core in range(NCORES):
        b = core // 2
        hg = core % 2
        in_maps.append({
            "xT": xTs[b],
            "wq": np.ascontiguousarray(
                w_qkv[hg * DH:(hg + 1) * DH, :].T).astype(BF),
            "wk": np.ascontiguousarray(
                w_qkv[C + hg * DH:C + (hg + 1) * DH, :].T).astype(BF),
            "wv": np.ascontiguousarray(
                w_qkv[2 * C + hg * DH:2 * C + (hg + 1) * DH, :].T).astype(BF),
            "wp": np.ascontiguousarray(
                w_proj[:, hg * DH:(hg + 1) * DH].T).astype(BF),
            "cos2": cos2,
            "sin2": sin2,
            "p2t": p2t,
        })
    return in_maps


def kernel(x, rope_cos, rope_sin, w_qkv, w_proj, b_proj, trace=False):
    from concourse.bass_utils import run_bass_kernel_spmd

    nc = build_nc()
    in_maps = make_in_maps(x, rope_cos, rope_sin, w_qkv, w_proj)
    res = run_bass_kernel_spmd(nc, in_maps, core_ids=list(range(NCORES)),
                               trace=trace)
    b_proj = np.asarray(b_proj, dtype=np.float32)
    final = np.empty((B, N, C), dtype=np.float32)
    for b in range(B):
        final[b] = res.results[2 * b]["out"] + res.results[2 * b + 1]["out"] \
            + b_proj
    if trace:
        kernel.last_exec_time_ns = res.exec_time_ns
        kernel.last_results = res
    return final


# revision 42
# speedup vs baseline: 1.3233x; 1.0155x over previous
"""Fused multi-head attention (qkv + RoPE + softmax + proj) for TRN2, 8 cores.

Sharding: core c -> batch b=c//2, head group hg=c%2 (8 of 16 heads).
Data-parallel over B (4), 2-way tensor-parallel over heads.
Host unshard: out[b] = partial[2b] + partial[2b+1] + b_proj.

v4: all matmul operands bf16 (1 cyc/col @2.4GHz).  The attention loop is
gated by the ScalarE softmax exp at ~(1024+340)/1.2GHz ~= 1.12us per key
tile; everything else is paced to hide under it:
  head:  k^T (+rope; the rot matmul is delayed one tile so the PE never
         waits on the psum->sbuf copy), v, q^T chunk 0.
  loop:  scores pair (tile_position quadrants) -> exp -> attn@v pair, plus
         a filler queue popped once per ki at 1-matmul granularity:
         remaining q^T tiles, proj tiles, and the block-boundary softmax
         normalization (reciprocal on DVE, partition-broadcast via a
         [1,64] ones matmul, psum x psum multiply into aoT on DVE).
PSUM budget: scores 2x2 banks, attn@v accumulators 2 (single-buffered),
filler accum 1, rot/broadcast 1.
"""

import sys

if "/opt/trn_rl_repo" not in sys.path:
    sys.path.insert(0, "/opt/trn_rl_repo")

import numpy as np
from contextlib import ExitStack

B, N, C, H, D = 4, 2048, 1024, 16, 64
NCORES = 8
P = 128
DH = 512          # per-core head channels (8 heads x 64)
CT = C // P       # 8 contraction tiles for qkv
DHT = DH // P     # 4 partition tiles of qT/kT/aoT
NT = N // P       # 16 n tiles
NCH = N // 512    # 4 n chunks of 512
KT = N // P       # 16 key tiles

_CACHE = {}


def _emit(nc, tc, mybir, bass, tile):
    F32 = mybir.dt.float32
    BF16 = mybir.dt.bfloat16
    FP16 = mybir.dt.float16
    Exp = mybir.ActivationFunctionType.Exp

    xT = nc.dram_tensor("xT", [C, N], BF16, kind="ExternalInput").ap()
    wq = nc.dram_tensor("wq", [C, DH], BF16, kind="ExternalInput").ap()
    wk = nc.dram_tensor("wk", [C, DH], BF16, kind="ExternalInput").ap()
    wv = nc.dram_tensor("wv", [C, DH], BF16, kind="ExternalInput").ap()
    wp = nc.dram_tensor("wp", [DH, C], BF16, kind="ExternalInput").ap()
    cos2 = nc.dram_tensor("cos2", [P, N], BF16, kind="ExternalInput").ap()
    sin2 = nc.dram_tensor("sin2", [P, N], BF16, kind="ExternalInput").ap()
    p2t = nc.dram_tensor("p2t", [P, P], BF16, kind="ExternalInput").ap()
    out = nc.dram_tensor("out", [N, C], F32, kind="ExternalOutput").ap()

    def merged(src, rows, blocks, width, off=0):
        # [blocks*128, width] dram slab -> [128, blocks, width] sbuf tile
        return bass.AP(tensor=src.tensor, offset=off,
                       ap=[[rows, P], [P * rows, blocks], [1, width]])

    ctx = ExitStack()
    with ctx:
        consts = ctx.enter_context(tc.tile_pool(name="consts", bufs=1))
        persist = ctx.enter_context(tc.tile_pool(name="persist", bufs=1))

        cos_sb = consts.tile([P, N], BF16, tag="cos")
        sin_sb = consts.tile([P, N], BF16, tag="sin")
        p2t_sb = consts.tile([P, P], BF16, tag="p2t")
        qT = [persist.tile([P, N], BF16, tag=f"qT{t}", name=f"qT{t}")
              for t in range(DHT)]
        kTt = [persist.tile([P, N], BF16, tag=f"kT{t}", name=f"kT{t}")
               for t in range(DHT)]
        v_sb = [persist.tile([P, 8 * 65], BF16, tag=f"v{i}", name=f"v{i}")
                for i in range(NT)]
        # x and wq stay resident: attention-phase filler computes qT c1..c3
        x_sb = [persist.tile([P, CT, 512], BF16, tag=f"x{i}", name=f"x{i}")
                for i in range(NCH)]
        wq_sb = persist.tile([P, CT, 512], BF16, tag="wq", name="wq")
        wp_sb = persist.tile([P, DHT, C], BF16, tag="wp", name="wp")
        aoT = [persist.tile([P, N], BF16, tag=f"aoT{t}", name=f"aoT{t}")
               for t in range(DHT)]

        p1 = ExitStack()
        wpool = p1.enter_context(tc.tile_pool(name="wkv", bufs=1))
        tpool = p1.enter_context(tc.tile_pool(name="p1tmp", bufs=3))
        qk_ps_pool = p1.enter_context(
            tc.tile_pool(name="p1ps", bufs=3, space="PSUM"))
        rot_ps_pool = p1.enter_context(
            tc.tile_pool(name="p1ps2", bufs=2, space="PSUM"))
        v_ps_pool = p1.enter_context(
            tc.tile_pool(name="p1ps3", bufs=2, space="PSUM"))

        wk_sb = wpool.tile([P, CT, 512], BF16, tag="wk", name="wk")
        wv_sb = wpool.tile([P, CT, 512], BF16, tag="wv", name="wv")

        # ---- DMA issue order: first compute needs x0+wk ----
        def load_x(nch):
            for g in range(4):
                sub = bass.AP(tensor=xT.tensor,
                              offset=(g * 2 * P) * N + nch * 512,
                              ap=[[N, P], [P * N, 2], [1, 512]])
                nc.sync.dma_start(x_sb[nch][:, 2 * g:2 * g + 2], sub)
        def load_w(dst, src, pairs=4, width=DH):
            # kc-pair sub-DMAs: short issue, parallel engines, and the
            # first accumulation matmuls only need the first pair
            for g in range(pairs):
                sub = bass.AP(tensor=src.tensor, offset=(g * 2 * P) * width,
                              ap=[[width, P], [P * width, 2], [1, width]])
                nc.sync.dma_start(dst[:, 2 * g:2 * g + 2], sub)
        def load_w_g(dst, src, g, width=DH):
            sub = bass.AP(tensor=src.tensor, offset=(g * 2 * P) * width,
                          ap=[[width, P], [P * width, 2], [1, width]])
            nc.sync.dma_start(dst[:, 2 * g:2 * g + 2], sub)

        def load_x_g(nch, g):
            sub = bass.AP(tensor=xT.tensor,
                          offset=(g * 2 * P) * N + nch * 512,
                          ap=[[N, P], [P * N, 2], [1, 512]])
            nc.sync.dma_start(x_sb[nch][:, 2 * g:2 * g + 2], sub)

        for g in range(4):   # interleave: first matmuls need wk+x0 pair 0
            load_w_g(wk_sb, wk, g)
            load_x_g(0, g)
        nc.sync.dma_start(p2t_sb, p2t)
        load_w(wq_sb, wq)
        load_x(1)
        nc.sync.dma_start(cos_sb, cos2)
        nc.sync.dma_start(sin_sb, sin2)
        load_x(2)
        load_w(wv_sb, wv)
        load_x(3)
        load_w(wp_sb, wp, pairs=2, width=C)
        for i in range(NT):   # softmax-denominator ones column of v
            ones_cols = bass.AP(
                tensor=v_sb[i].tensor, offset=64,
                ap=[list(v_sb[i].ap[0]), [65, 8]])
            nc.vector.memset(ones_cols, 1.0)

        # ---- phase 1 head: k (+rope, rot delayed 1 tile), v, q chunk 0 ----
        rot_pend = []

        def emit_rot():
            raw, dst, nsl = rot_pend.pop(0)
            rot = rot_ps_pool.tile([P, 512], F32, tag="rot_ps")
            nc.tensor.matmul(rot, p2t_sb, raw, start=True, stop=True)
            t1 = tpool.tile([P, 512], F32, tag="t1")
            nc.vector.tensor_mul(t1, raw, cos_sb[:, nsl])
            t2 = tpool.tile([P, 512], F32, tag="t2")
            nc.vector.tensor_mul(t2, rot, sin_sb[:, nsl])
            nc.vector.tensor_add(dst[:, nsl], t1, t2)

        def emit_qk_tile(w_sb, dst, t, nch):
            nsl = slice(nch * 512, (nch + 1) * 512)
            ps = qk_ps_pool.tile([P, 512], F32, tag="qk_ps")
            for kc in range(CT):
                nc.tensor.matmul(
                    ps, w_sb[:, kc, t * P:(t + 1) * P], x_sb[nch][:, kc],
                    start=(kc == 0), stop=(kc == CT - 1))
            raw = tpool.tile([P, 512], BF16, tag="raw")
            nc.scalar.copy(raw, ps)
            rot_pend.append((raw, dst, nsl))
            if len(rot_pend) > 1:
                emit_rot()

        for nch in range(NCH):
            for t in range(DHT):
                emit_qk_tile(wk_sb, kTt[t], t, nch)
        for nch in range(NCH):
            for nt4 in range(4):
                i = nch * 4 + nt4
                ps = v_ps_pool.tile([P, 512], F32, tag="v_ps")
                for kc in range(CT):
                    nc.tensor.matmul(
                        ps, x_sb[nch][:, kc, nt4 * P:(nt4 + 1) * P],
                        wv_sb[:, kc],
                        start=(kc == 0), stop=(kc == CT - 1))
                v_view = bass.AP(
                    tensor=v_sb[i].tensor, offset=0,
                    ap=[list(v_sb[i].ap[0]), [65, 8], [1, 64]])
                nc.scalar.copy(v_view, ps.rearrange("p (h d) -> p h d", h=8))
        for t in range(DHT):   # q chunk 0 only; chunks 1-3 fill attention
            emit_qk_tile(wq_sb, qT[t], t, 0)
        while rot_pend:
            emit_rot()
        p1.close()

        # ---------------- attention + proj ----------------
        attn_ctx = ExitStack()
        epool = attn_ctx.enter_context(tc.tile_pool(name="epool2", bufs=5))
        atmp = attn_ctx.enter_context(tc.tile_pool(name="atmp", bufs=3))
        ftmp = attn_ctx.enter_context(tc.tile_pool(name="ftmp", bufs=2))
        s_ps_pool = attn_ctx.enter_context(
            tc.tile_pool(name="s_ps", bufs=1, space="PSUM"))
        o_ps_pool = attn_ctx.enter_context(
            tc.tile_pool(name="o_ps", bufs=1, space="PSUM"))
        f_ps_pool = attn_ctx.enter_context(
            tc.tile_pool(name="f_ps", bufs=2, space="PSUM"))
        pending = []   # filler pieces: (closure, tag)

        def make_proj_pieces(nt, fc):
            # one output tile's proj as two 2-matmul PE filler pieces; the
            # psum tile is created by piece A and finished by piece B
            box = {}

            def pa():
                ps = f_ps_pool.tile([P, 512], F32, tag="fp",
                                    name="pps")
                box["ps"] = ps
                for ct in range(2):
                    nc.tensor.matmul(
                        ps, aoT[ct][:, nt * P:(nt + 1) * P],
                        wp_sb[:, ct, fc * 512:(fc + 1) * 512],
                        start=(ct == 0), stop=False)

            def pb():
                ps = box["ps"]
                for ct in range(2, DHT):
                    nc.tensor.matmul(
                        ps, aoT[ct][:, nt * P:(nt + 1) * P],
                        wp_sb[:, ct, fc * 512:(fc + 1) * 512],
                        start=False, stop=(ct == DHT - 1))
                ob = atmp.tile([P, 512], F32, tag="ob")
                nc.vector.tensor_copy(ob, ps)
                nc.sync.dma_start(
                    out[nt * P:(nt + 1) * P, fc * 512:(fc + 1) * 512], ob)
            return [(pa, "p"), (pb, "p")]

        def make_q_pieces(t, nch):
            # qT tile for chunk nch as filler pieces (2 matmuls each);
            # psum from the dedicated filler pool, copies on DVE
            nsl = slice(nch * 512, (nch + 1) * 512)
            box = {}

            def qm(k2):
                def fn():
                    if k2 == 0:
                        box["ps"] = f_ps_pool.tile([P, 512], F32, tag="fp",
                                                   name="qf")
                    for kc in (2 * k2, 2 * k2 + 1):
                        nc.tensor.matmul(
                            box["ps"], wq_sb[:, kc, t * P:(t + 1) * P],
                            x_sb[nch][:, kc],
                            start=(kc == 0), stop=(kc == CT - 1))
                return fn

            def qraw():
                raw = ftmp.tile([P, 512], BF16, tag="fraw")
                box["raw"] = raw
                nc.vector.tensor_copy(raw, box["ps"])

            def qrot():
                rot = f_ps_pool.tile([P, 512], F32, tag="fp", name="qr")
                box["rot"] = rot
                nc.tensor.matmul(rot, p2t_sb, box["raw"], start=True,
                                 stop=True)
                t1 = ftmp.tile([P, 512], F32, tag="ft1")
                box["t1"] = t1
                nc.vector.tensor_mul(t1, box["raw"], cos_sb[:, nsl])

            def qt2():
                t2 = ftmp.tile([P, 512], F32, tag="ft2")
                nc.vector.tensor_mul(t2, box["rot"], sin_sb[:, nsl])
                nc.vector.tensor_add(qT[t][:, nsl], box["t1"], t2)
            return [(qm(0), ("q", nch)), (qm(1), ("q", nch)),
                    (qm(2), ("q", nch)), (qm(3), ("q", nch)),
                    (qraw, ("q", nch)), (qrot, ("q", nch)),
                    (qt2, ("q", nch))]

        def emit_norm(o_ps_par, hp, qsl, pb_):
            dd = atmp.tile([1, 512], F32, tag="dd")
            nc.vector.tensor_copy(dd, o_ps_par[64:65, :])
            r32 = atmp.tile([1, 512], F32, tag="r32")
            nc.vector.reciprocal_approx_fast(r32, dd)
            rb = atmp.tile([P, 512], F32, tag="rb")
            nc.gpsimd.partition_broadcast(rb[0:64, :], r32, channels=64)
            nc.vector.tensor_mul(
                aoT[hp][pb_:pb_ + 64, qsl], o_ps_par[0:64, :],
                rb[0:64, :])

        def drain_q_chunk(nch):
            # correctness: qT chunk writers must be emitted before the
            # first scores read of that chunk (deps follow emission order)
            while any(tag == ("q", nch) for _, tag in pending):
                pending.pop(0)[0]()

        for nch in range(1, NCH):
            for t in range(DHT):
                pending.extend(make_q_pieces(t, nch))

        for qc in range(NCH):
            qsl = slice(qc * 512, (qc + 1) * 512)
            if qc > 0:
                drain_q_chunk(qc)
            for hp in range(4):      # head pairs (even@part0-63, odd@64-127)
                o_ps = {}
                for par in range(2):  # par=0: even head, par=1: odd head
                    o_ps[par] = o_ps_pool.tile([P, 512], F32,
                                               tag=f"o{par}", name=f"o{par}")

                def emit_av(ki, e):
                    for par in range(2):
                        h = hp * 2 + par
                        # rows 0-63: attn@v; row 64: softmax denominator
                        nc.tensor.matmul(
                            o_ps[par][0:65, :],
                            v_sb[ki][:, h * 65:(h + 1) * 65],
                            e[:, par],
                            start=(ki == 0), stop=(ki == KT - 1))

                # attn@v lags 4 ki behind scores so the previous block's
                # normalize chain finishes before av(ki=0) clears the
                # single-buffered o_ps bank
                prevs = []
                for ki in range(KT):
                    ksl = slice(ki * P, (ki + 1) * P)
                    s_ps = s_ps_pool.tile([P, 2, 512], F32,
                                          tag=f"s{ki % 2}", name=f"s{ki % 2}")
                    for par in range(2):
                        pb_ = par * 64
                        nc.tensor.matmul(
                            s_ps[:, par],
                            kTt[hp][pb_:pb_ + 64, ksl],
                            qT[hp][pb_:pb_ + 64, qsl],
                            start=True, stop=True,
                            tile_position=(pb_, 0))
                    if len(prevs) >= 4:
                        emit_av(*prevs.pop(0))
                    # keep a small reserve so dependency-free pieces remain
                    # to bridge the end-of-loop normalize chain (HAM-warm)
                    if len(pending) > 4 and (ki % 2 == 1 or ki % 4 == 0):
                        pending.pop(0)[0]()
                    e = epool.tile([P, 2, 512], BF16, tag="e", name="e")
                    nc.scalar.activation(e, s_ps, Exp,
                                         scale=float(D) ** -0.5)
                    prevs.append((ki, e))
                for p in prevs:
                    emit_av(*p)
                for par in range(2):
                    emit_norm(o_ps[par], hp, qsl, par * 64)
            for nt in range(qc * 4, (qc + 1) * 4):
                for fc in range(2):
                    pending.extend(make_proj_pieces(nt, fc))
        for fn, _ in pending:
            fn()
        attn_ctx.close()


def build_nc():
    if "nc" in _CACHE:
        return _CACHE["nc"]
    import concourse.bass as bass
    import concourse.tile as tile
    from concourse import bacc, mybir

    nc = bacc.Bacc("TRN2", target_bir_lowering=False, debug=False,
                   enable_asserts=False, num_devices=NCORES)
    with tile.TileContext(nc) as tc:
        _emit(nc, tc, mybir, bass, tile)
    nc.compile()
    _CACHE["nc"] = nc
    return nc


def make_in_maps(x, rope_cos, rope_sin, w_qkv, w_proj):
    import ml_dtypes
    BF = ml_dtypes.bfloat16

    x = np.asarray(x, dtype=np.float32)
    rope_cos = np.asarray(rope_cos, dtype=np.float32)
    rope_sin = np.asarray(rope_sin, dtype=np.float32)
    w_qkv = np.asarray(w_qkv, dtype=np.float32)
    w_proj = np.asarray(w_proj, dtype=np.float32)

    cosT = np.ascontiguousarray(rope_cos.T)           # [64, N]
    cos2 = np.vstack([cosT, cosT]).astype(BF)         # [128, N]
    sinT = np.ascontiguousarray(rope_sin.T)
    sin2 = np.vstack([sinT, sinT]).astype(BF)

    # signed half-rotation permutation: rot(q) = P2 @ q (per 64-block)
    R = np.zeros((D, D), dtype=np.float32)
    half = D // 2
    R[np.arange(half), np.arange(half) + half] = -1.0
    R[np.arange(half) + half, np.arange(half)] = 1.0
    P2 = np.zeros((P, P), dtype=np.float32)
    P2[:D, :D] = R
    P2[D:, D:] = R
    p2t = np.ascontiguousarray(P2.T).astype(BF)

    xTs = [np.ascontiguousarray(x[b].T).astype(BF) for b in range(B)]

    in_maps = []
    for core in range(NCORES):
        b = core // 2
        hg = core % 2
        in_maps.append({
            "xT": xTs[b],
            "wq": np.ascontiguousarray(
                w_qkv[hg * DH:(hg + 1) * DH, :].T).astype(BF),
            "wk": np.ascontiguousarray(
                w_qkv[C + hg * DH:C + (hg + 1) * DH, :].T).astype(BF),
            "wv": np.ascontiguousarray(
                w_qkv[2 * C + hg * DH:2 * C + (hg + 1) * DH, :].T).astype(BF),
            "wp": np.ascontiguousarray(
                w_proj[:, hg * DH:(hg + 1) * DH].T).astype(BF),
            "cos2": cos2,
            "sin2": sin2,
            "p2t": p2t,
        })
    return in_maps


def kernel(x, rope_cos, rope_sin, w_qkv, w_proj, b_proj, trace=False):
    from concourse.bass_utils import run_bass_kernel_spmd

    nc = build_nc()
    in_maps = make_in_maps(x, rope_cos, rope_sin, w_qkv, w_proj)
    res = run_bass_kernel_spmd(nc, in_maps, core_ids=list(range(NCORES)),
                               trace=trace)
    b_proj = np.asarray(b_proj, dtype=np.float32)
    final = np.empty((B, N, C), dtype=np.float32)
    for b in range(B):
        final[b] = res.results[2 * b]["out"] + res.results[2 * b + 1]["out"] \
            + b_proj
    if trace:
        kernel.last_exec_time_ns = res.exec_time_ns
        kernel.last_results = res
    return final
